# revision 1
# baseline (speedup 1.0000x reference)
"""Causal self-attention (RMSNorm-QK + RoPE) Trainium2 Bass kernel.

Problem: B=2, T=2048, C=1024, H=16 heads, D=64.
Sharding: 8 cores = 2 (batch) x 4 (head groups of 4 heads).
Each core computes q/k/v projections for its 4 heads, attention, and a
partial output projection (column-parallel over heads); the host sums the
4 partials per batch and transposes.

All matmuls run in float32r (TF32-like, ~13-bit mantissa, 4x fp32 matmul
speed). f32r matmul operands must be produced by rounding ops or f32r DMA;
host pre-rounds the DRAM inputs.

Per-core layouts ("T-layout" = channels on partitions, tokens free):
  projection chunks [128, 512]: row 32h+i = head h, rope-half dim i
  qT_r/kT_r  2 x [128, 2048] f32r : chunk c rows 64*(h%2)+d = head 2c+h%2
  v_r        16 x [128, 260] f32r : head h at cols 65h..65h+63, ones col
  scoresT    [s-chunk 128, t-block 512]; softmax denom = ones-column row
  yT_sb      2 x [128, 2048] f32r : pair chunk c = heads (2c, 2c+1)
Output: outT [1024, 2048] = (partial out).T per core; host sums + transposes.
"""

import sys

for _p in ("/opt/trn_rl_repo",):
    if _p not in sys.path:
        sys.path.append(_p)

import numpy as np

B, T, C = 2, 2048, 1024
H_TOT, D = 16, 64
HPC = 4               # heads per core
N_CORES = 8
P = 128               # partitions
NB = 4                # t-blocks of 512
TB = 512              # t-block size
KCH = 8               # C / 128 contraction chunks
VW = 65 * HPC         # v width with ones columns = 260
RMS_EPS = 1.1920928955078125e-07
ROPE_BASE = 10000.0

_CACHE = {}


def _build_consts():
    """Host-side constant tensors shared by all cores."""
    inv_freq = (1.0 / (ROPE_BASE ** (np.arange(0, D, 2, dtype=np.float32) / np.float32(D)))).astype(np.float32)
    pos = np.arange(T, dtype=np.float32)
    freqs = np.outer(pos, inv_freq).astype(np.float32)      # [T, 32]
    cos = np.cos(freqs).astype(np.float32)                  # [T, 32]
    sin = np.sin(freqs).astype(np.float32)
    cosr = np.ascontiguousarray(np.tile(cos.T, (HPC, 1)))   # [128, T]
    sinr = np.ascontiguousarray(np.tile(sin.T, (HPC, 1)))
    # ind32 [128, 4]: per-32-row-group summing matrix (lhsT for RMS sums)
    ind32 = np.zeros((P, HPC), dtype=np.float32)
    for p_ in range(P):
        ind32[p_, p_ // 32] = 1.0
    # bc32 [4, 128]: broadcast inv (4 heads) to 32-row groups (lhsT)
    bc32 = np.zeros((HPC, P), dtype=np.float32)
    for p_ in range(P):
        bc32[p_ // 32, p_] = 1.0
    # selpair [128, 256]: chunk c (=0,1): col m -> den row 32*(2c + m//64)
    selpair = np.zeros((P, 2 * P), dtype=np.float32)
    for c in range(2):
        for m in range(P):
            selpair[32 * (2 * c + m // 64), 128 * c + m] = 1.0
    return dict(cosr=cosr, sinr=sinr, ind32=ind32, bc32=bc32,
                selpair=selpair)


def _build_module():
    import concourse.bacc as bacc
    import concourse.mybir as mybir
    import concourse.tile as tile

    f32 = mybir.dt.float32
    f32r = mybir.dt.float32r
    Exp = mybir.ActivationFunctionType.Exp
    Ln = mybir.ActivationFunctionType.Ln
    Alu = mybir.AluOpType

    nc = bacc.Bacc("TRN2", target_bir_lowering=False, debug=False,
                   num_devices=N_CORES)

    xt_d = nc.dram_tensor("xt", [C, T], f32r, kind="ExternalInput").ap()
    wq_d = nc.dram_tensor("wq", [C, 256], f32r, kind="ExternalInput").ap()
    wk_d = nc.dram_tensor("wk", [C, 256], f32r, kind="ExternalInput").ap()
    wv_d = nc.dram_tensor("wv", [C, VW], f32r, kind="ExternalInput").ap()
    wp_d = nc.dram_tensor("wp", [256, C], f32r, kind="ExternalInput").ap()
    cosr_d = nc.dram_tensor("cosr", [P, T], f32, kind="ExternalInput").ap()
    sinr_d = nc.dram_tensor("sinr", [P, T], f32, kind="ExternalInput").ap()
    ind32_d = nc.dram_tensor("ind32", [P, HPC], f32r, kind="ExternalInput").ap()
    bc32_d = nc.dram_tensor("bc32", [HPC, P], f32r, kind="ExternalInput").ap()
    selpair_d = nc.dram_tensor("selpair", [P, 2 * P], f32r, kind="ExternalInput").ap()
    zeros_d = nc.dram_tensor("zeros", [64, T], f32r, kind="ExternalInput").ap()
    out_d = nc.dram_tensor("outT", [C, T], f32, kind="ExternalOutput").ap()

    with tile.TileContext(nc) as tc:
        with (
            tc.tile_pool(name="sb", bufs=1) as sb,
            tc.tile_pool(name="trans", bufs=2) as tr,
            tc.tile_pool(name="ps", bufs=2, space="PSUM") as ps,
        ):
            # ---- constants / weights in (direct f32r DMA) ----
            def direct_load(name, dram_slice, shape, dt=f32r):
                t_r = sb.tile(shape, dt, tag=name, name=name)
                nc.sync.dma_start(out=t_r[:], in_=dram_slice)
                return t_r

            ind32_r = direct_load("ind32r", ind32_d[:, :], [P, HPC])
            bc32_r = direct_load("bc32r", bc32_d[:, :], [HPC, P])
            selpair_r = direct_load("selpairr", selpair_d[:, :], [P, 2 * P])
            cosr_t = direct_load("cosr", cosr_d[:, :], [P, T], f32)
            sinr_t = direct_load("sinr", sinr_d[:, :], [P, T], f32)
            wq_r = [direct_load(f"wqr{k}", wq_d[k * P:(k + 1) * P, :], [P, 256])
                    for k in range(KCH)]
            wk_r = [direct_load(f"wkr{k}", wk_d[k * P:(k + 1) * P, :], [P, 256])
                    for k in range(KCH)]
            wv_r = [direct_load(f"wvr{k}", wv_d[k * P:(k + 1) * P, :], [P, VW])
                    for k in range(KCH)]
            wp_r = [direct_load(f"wpr{c}", wp_d[c * P:(c + 1) * P, :], [P, C])
                    for c in range(2)]

            # ---- persistent intermediates ----
            qT_r = [sb.tile([P, T], f32r, tag=f"qT{c}", name=f"qT{c}")
                    for c in range(2)]
            # kTe[c]: rows 0-63 = head 2c, rows 64-127 zero;
            # kTo[c]: rows 0-63 zero, rows 64-127 = head 2c+1.
            kTe = [sb.tile([P, T], f32r, tag=f"kTe{c}", name=f"kTe{c}")
                   for c in range(2)]
            kTo = [sb.tile([P, T], f32r, tag=f"kTo{c}", name=f"kTo{c}")
                   for c in range(2)]
            for c in range(2):
                nc.sync.dma_start(out=kTe[c][64:128, :], in_=zeros_d[:, :])
                nc.sync.dma_start(out=kTo[c][0:64, :], in_=zeros_d[:, :])
            v_r = [sb.tile([P, VW], f32r, tag=f"v{s}", name=f"v{s}")
                   for s in range(T // P)]
            yT_sb = [sb.tile([P, T], f32r, tag=f"yT{c}", name=f"yT{c}")
                     for c in range(2)]
            den_stack = sb.tile([P, T], f32r, tag="denstack", name="den_stack")
            nc.gpsimd.memset(den_stack[:].bitcast(f32), 1.0)
            eps_t = sb.tile([HPC, 1], f32, tag="epst", name="eps_t")
            nc.gpsimd.memset(eps_t[:], RMS_EPS)

            # ====== Phase 1+2: projections, RMS-norm, RoPE, repack ======
            for n in range(NB):
                nsl = slice(n * TB, (n + 1) * TB)
                xr_t = []
                for k in range(KCH):
                    xr = tr.tile([P, TB], f32r, tag="xr", name=f"xr{n}_{k}", bufs=9)
                    nc.sync.dma_start(out=xr[:], in_=xt_d[k * P:(k + 1) * P, nsl])
                    xr_t.append(xr)
                pq0 = ps.tile([P, TB], f32, tag="psA", name=f"pq0_{n}")
                pq1 = ps.tile([P, TB], f32, tag="psA", name=f"pq1_{n}")
                pk0 = ps.tile([P, TB], f32, tag="psB", name=f"pk0_{n}")
                pk1 = ps.tile([P, TB], f32, tag="psB", name=f"pk1_{n}")
                for k in range(KCH):
                    xr = xr_t[k]
                    st = (k == 0)
                    sp = (k == KCH - 1)
                    nc.tensor.matmul(pq0[:], lhsT=wq_r[k][:, 0:128], rhs=xr[:],
                                     start=st, stop=sp)
                    nc.tensor.matmul(pq1[:], lhsT=wq_r[k][:, 128:256], rhs=xr[:],
                                     start=st, stop=sp)
                    nc.tensor.matmul(pk0[:], lhsT=wk_r[k][:, 0:128], rhs=xr[:],
                                     start=st, stop=sp)
                    nc.tensor.matmul(pk1[:], lhsT=wk_r[k][:, 128:256], rhs=xr[:],
                                     start=st, stop=sp)
                # q/k chunks out of PSUM
                x1q = tr.tile([P, TB], f32, tag="x1q", name=f"x1q{n}", bufs=1)
                x2q = tr.tile([P, TB], f32, tag="x2q", name=f"x2q{n}", bufs=1)
                x1k = tr.tile([P, TB], f32, tag="x1k", name=f"x1k{n}", bufs=1)
                x2k = tr.tile([P, TB], f32, tag="x2k", name=f"x2k{n}", bufs=1)
                nc.vector.tensor_copy(x1q[:], pq0[:])
                nc.vector.tensor_copy(x2q[:], pq1[:])
                nc.vector.tensor_copy(x1k[:], pk0[:])
                nc.vector.tensor_copy(x2k[:], pk1[:])
                # v projections (second sub-pass over the same xr tiles)
                pv = [ps.tile([P, VW], f32, tag=("psA" if s < 2 else "psB"),
                              name=f"pv{n}_{s}") for s in range(4)]
                for k in range(KCH):
                    st = (k == 0)
                    sp = (k == KCH - 1)
                    for s_rel in range(4):
                        nc.tensor.matmul(
                            pv[s_rel][:],
                            lhsT=xr_t[k][:, s_rel * P:(s_rel + 1) * P],
                            rhs=wv_r[k][:], start=st, stop=sp)
                for s_rel in range(4):
                    vt = v_r[4 * n + s_rel]
                    nc.vector.tensor_copy(vt[:], pv[s_rel][:])
                    nc.vector.tensor_scalar(vt[:, 64:VW:65], pv[s_rel][:, 64:VW:65],
                                            0.0, 1.0, Alu.mult, Alu.add)
                # RMS-norm + RoPE + repack, per tensor
                for (x1, x2, dstT, eng) in ((x1q, x2q, qT_r, "q"),
                                            (x1k, x2k, None, "k")):
                    e = nc.vector if eng == "q" else nc.gpsimd
                    sq1 = tr.tile([P, TB], f32r, tag="tmpA", name=f"sq1{eng}{n}", bufs=1)
                    sq2 = tr.tile([P, TB], f32r, tag="tmpB", name=f"sq2{eng}{n}", bufs=1)
                    nc.gpsimd.tensor_mul(sq1[:], x1[:], x1[:])
                    nc.gpsimd.tensor_mul(sq2[:], x2[:], x2[:])
                    ps_s = ps.tile([HPC, TB], f32, tag="psA", name=f"pss{eng}{n}")
                    nc.tensor.matmul(ps_s[:], lhsT=ind32_r[:], rhs=sq1[:],
                                     start=True, stop=False)
                    nc.tensor.matmul(ps_s[:], lhsT=ind32_r[:], rhs=sq2[:],
                                     start=False, stop=True)
                    invc = tr.tile([HPC, TB], f32r, tag="invc", name=f"invc{eng}{n}")
                    nc.scalar.activation(invc[:], ps_s[:], Ln,
                                         bias=eps_t[:], scale=1.0 / 64.0)
                    nc.scalar.activation(invc[:], invc[:], Exp, scale=-0.5)
                    ps_b = ps.tile([P, TB], f32, tag="psB", name=f"psb{eng}{n}")
                    nc.tensor.matmul(ps_b[:], lhsT=bc32_r[:], rhs=invc[:],
                                     start=True, stop=True)
                    nc.vector.tensor_mul(x1[:], x1[:], ps_b[:])
                    nc.vector.tensor_mul(x2[:], x2[:], ps_b[:])
                    # rope
                    m_a = tr.tile([P, TB], f32, tag="tmpA", name=f"ma{eng}{n}", bufs=1)
                    m_b = tr.tile([P, TB], f32, tag="tmpB", name=f"mb{eng}{n}", bufs=1)
                    rc1 = tr.tile([P, TB], f32r, tag="roch1", name=f"rc1{eng}{n}", bufs=1)
                    rc2 = tr.tile([P, TB], f32r, tag="roch2", name=f"rc2{eng}{n}", bufs=1)
                    e.tensor_mul(m_a[:], x1[:], cosr_t[:, nsl])
                    e.tensor_mul(m_b[:], x2[:], sinr_t[:, nsl])
                    e.tensor_add(rc1[:], m_a[:], m_b[:])
                    m_c = tr.tile([P, TB], f32, tag="tmpA", name=f"mc{eng}{n}", bufs=1)
                    m_d = tr.tile([P, TB], f32, tag="tmpB", name=f"md{eng}{n}", bufs=1)
                    e.tensor_mul(m_c[:], x2[:], cosr_t[:, nsl])
                    e.tensor_mul(m_d[:], x1[:], sinr_t[:, nsl])
                    e.tensor_sub(rc2[:], m_c[:], m_d[:])
                    # repack: head h rows 32h..32h+32 of (rc1|rc2) ->
                    # q: qT_r[h//2] rows 64*(h%2)..; k: kTe/kTo (zero-padded)
                    for h in range(HPC):
                        if eng == "q":
                            dst = dstT[h // 2]
                            rb = 64 * (h % 2)
                        else:
                            dst = (kTe if h % 2 == 0 else kTo)[h // 2]
                            rb = 64 * (h % 2)
                        hs = slice(32 * h, 32 * h + 32)
                        nc.sync.dma_start(out=dst[rb:rb + 32, nsl], in_=rc1[hs, :])
                        nc.sync.dma_start(out=dst[rb + 32:rb + 64, nsl], in_=rc2[hs, :])

            # ================= Phase 3: attention =================
            for h in range(HPC):
                cch = h // 2
                kT_h = (kTe if h % 2 == 0 else kTo)[cch]
                rsl = slice(64 * (h % 2), 64 * (h % 2) + 64)
                pa = "psC"
                ya = "psD"
                et_tag = "expT" if h % 2 == 0 else "expT2"
                for j in range(NB):
                    jsl = slice(j * TB, (j + 1) * TB)
                    n_k = 4 * j + 4
                    Yh = ps.tile([65, TB], f32, tag=ya, name=f"Y{h}_{j}")
                    for k in range(n_k):
                        ksl = slice(k * P, (k + 1) * P)
                        st, sp = (k == 0), (k == n_k - 1)
                        r = k - 4 * j          # >=0 on diagonal blocks
                        # cols t < 128r of this block are fully masked; trim
                        # matmuls to N>=256 (f32r full-rate) and exp always.
                        mtrim = 128 * r if 0 < r <= 2 else 0
                        etrim = 128 * r if r > 0 else 0
                        msl = slice(mtrim, TB)
                        esl = slice(etrim, TB)
                        S0 = ps.tile([P, TB], f32, tag=pa, name=f"S{h}_{j}_{k}")
                        nc.tensor.matmul(S0[:, msl], lhsT=kT_h[:, ksl],
                                         rhs=qT_r[cch][:, j * TB + mtrim:(j + 1) * TB],
                                         start=True, stop=True)
                        e0 = tr.tile([P, TB], f32r, tag=et_tag,
                                     name=f"e{h}_{j}_{k}", bufs=3)
                        nc.scalar.activation(e0[:, esl], S0[:, esl], Exp, scale=0.125)
                        if r >= 0:  # diagonal: apply causal mask (zero-fills left)
                            e0m = tr.tile([P, TB], f32r, tag=et_tag,
                                          name=f"em{h}_{j}_{k}", bufs=3)
                            nc.gpsimd.affine_select(
                                out=e0m[:], in_=e0[:], pattern=[[1, TB]],
                                compare_op=Alu.is_ge, fill=0.0,
                                base=-128 * r, channel_multiplier=-1)
                            e0 = e0m
                        nc.tensor.matmul(Yh[:, msl], lhsT=v_r[k][:, 65 * h:65 * h + 65],
                                         rhs=e0[:, msl], start=st, stop=sp)
                    # copy out: y rows + den row (SBUF bounce; DMA shifts rows)
                    yb = tr.tile([65, TB], f32r, tag="cpbuf", name=f"yb{h}_{j}",
                                 bufs=3, padded_shape=[P, TB])
                    nc.vector.tensor_copy(yb[:], Yh[:])
                    nc.sync.dma_start(out=yT_sb[cch][rsl, jsl], in_=yb[0:64, :])
                    nc.sync.dma_start(out=den_stack[32 * h:32 * h + 1, jsl],
                                      in_=yb[64:65, :])

            # ================= Phase 4: normalize + out-projection ======
            # invden = exp(-ln(den)) on rows 0,32,64,96 (others memset to 1)
            invden_r = sb.tile([P, T], f32r, tag="invden", name="invden_r")
            nc.scalar.activation(den_stack[:], den_stack[:], Ln)
            nc.scalar.activation(invden_r[:], den_stack[:], Exp, scale=-1.0)
            for c in range(2):
                for n in range(NB):
                    nsl = slice(n * TB, (n + 1) * TB)
                    ps_i = ps.tile([P, TB], f32, tag="psA", name=f"psi{c}{n}")
                    nc.tensor.matmul(ps_i[:], lhsT=selpair_r[:, c * P:(c + 1) * P],
                                     rhs=invden_r[:, nsl], start=True, stop=True)
                    nc.vector.tensor_mul(yT_sb[c][:, nsl], yT_sb[c][:, nsl], ps_i[:])
            for o in range(8):
                osl = slice(o * P, (o + 1) * P)
                for n in range(NB):
                    nsl = slice(n * TB, (n + 1) * TB)
                    po = ps.tile([P, TB], f32, tag="psB", name=f"po{o}_{n}")
                    nc.tensor.matmul(po[:], lhsT=wp_r[0][:, osl], rhs=yT_sb[0][:, nsl],
                                     start=True, stop=False)
                    nc.tensor.matmul(po[:], lhsT=wp_r[1][:, osl], rhs=yT_sb[1][:, nsl],
                                     start=False, stop=True)
                    ob = tr.tile([P, TB], f32, tag="cpbuf", name=f"ob{o}_{n}", bufs=3)
                    nc.vector.tensor_copy(ob[:], po[:])
                    nc.sync.dma_start(out=out_d[osl, nsl], in_=ob[:])

    nc.compile()
    return nc


def _get_module():
    if "nc" not in _CACHE:
        _CACHE["nc"] = _build_module()
        _CACHE["consts"] = _build_consts()
    return _CACHE["nc"], _CACHE["consts"]


def _round_f32r(a, bits=10):
    u = np.ascontiguousarray(a, dtype=np.float32).view(np.uint32).astype(np.uint64)
    u = (u + (1 << (bits - 1))) & ~np.uint64((1 << bits) - 1)
    return np.minimum(u, 0xFFFFFFFF).astype(np.uint32).view(np.float32)


def _core_inputs(x, w_q, w_k, w_v, w_proj, core):
    """Build the per-core input map (numpy, host-side sharding)."""
    b = core // 4
    g = core % 4
    heads = [4 * g + j for j in range(HPC)]

    xt = _round_f32r(np.ascontiguousarray(x[b].T))        # [C, T]

    perm = np.empty(256, dtype=np.int64)
    for m in range(128):
        perm[m] = 64 * heads[m // 32] + (m % 32)             # x1 half
        perm[128 + m] = 64 * heads[m // 32] + 32 + (m % 32)  # x2 half
    wq = _round_f32r(np.ascontiguousarray(w_q[perm, :].T))   # [C, 256]
    wk = _round_f32r(np.ascontiguousarray(w_k[perm, :].T))

    # v weights with zero columns at 65h+64 (device writes the ones there)
    wv_aug = np.zeros((C, VW), dtype=np.float32)
    for j in range(HPC):
        wv_aug[:, 65 * j:65 * j + 64] = w_v[64 * heads[j]:64 * heads[j] + 64, :].T
    wv = _round_f32r(wv_aug)

    vperm = np.empty(256, dtype=np.int64)
    for m in range(256):
        vperm[m] = 64 * heads[m // 64] + (m % 64)
    wp = _round_f32r(np.ascontiguousarray(w_proj[:, vperm].T))  # [256, C]

    zeros = np.zeros((64, T), dtype=np.float32)
    return dict(xt=xt, wq=wq, wk=wk, wv=wv, wp=wp, zeros=zeros)


def kernel(x, w_q, w_k, w_v, w_proj, _trace=False, _trace_cores=None):
    from concourse.bass_utils import run_bass_kernel_spmd

    nc, consts = _get_module()
    x = np.asarray(x, dtype=np.float32)
    in_maps = []
    for core in range(N_CORES):
        m = _core_inputs(np.asarray(x), np.asarray(w_q), np.asarray(w_k),
                         np.asarray(w_v), np.asarray(w_proj), core)
        m.update(consts)
        in_maps.append(m)

    res = run_bass_kernel_spmd(nc, in_maps, list(range(N_CORES)),
                               trace=_trace, trace_cores=_trace_cores)
    outs = [res.results[c]["outT"] for c in range(N_CORES)]
    out = np.empty((B, T, C), dtype=np.float32)
    for b in range(B):
        acc = outs[4 * b].astype(np.float32)
        for g in range(1, 4):
            acc = acc + outs[4 * b + g]
        out[b] = acc.T
    if _trace:
        kernel._last_exec_time_ns = res.exec_time_ns
        kernel._last_results = res
    return out



# revision 18
# speedup vs baseline: 1.0780x; 1.0780x over previous
"""Causal self-attention (RMSNorm-QK + RoPE) Trainium2 Bass kernel.

Problem: B=2, T=2048, C=1024, H=16 heads, D=64.
Sharding: 8 cores = 2 (batch) x 4 (head groups of 4 heads). Each core does
q/k/v projections for its 4 heads, attention, and a column-parallel partial
out-projection; host sums 4 bf16 partials per batch and transposes.

Key structure (vs. a naive port):
 - single activation-table set (natural_log_exp_and_others) preloaded: Ln,
   Exp and Square share one table -> zero ACT_TABLE_LOAD thrash.
 - scores matmuls 2-way row-tiled (K=64 per head, heads of a pair in rows
   0-63 / 64-127), attn*V and denominator matmuls col-tiled via
   tile_position -> PE array halves run concurrently.
 - softmax exp batched per head-pair [128, 1024] out of a 2-bank PSUM tile,
   trimmed to the causal range with a 3D access pattern.
 - denominator = col-tiled ones32 matmuls (32-row replicated), inverted with
   reciprocal_approx_fast on DVE, applied via a selpair matmul + a fused
   multiply on the Y PSUM->SBUF copy.
 - f32r on the q/k/scores path, bf16 on the v/probs/out path.
"""

import os
import sys

for _p in ("/opt/trn_rl_repo",):
    if _p not in sys.path:
        sys.path.append(_p)

import numpy as np

_KDBG = bool(os.environ.get("KDBG"))

B, T, C = 2, 2048, 1024
H_TOT, D = 16, 64
HPC = 4               # heads per core
N_CORES = 8
P = 128               # partitions
NB = 4                # t-blocks of 512
TB = 512              # t-block size
KCH = 8               # C / 128 contraction chunks
RMS_EPS = 1.1920928955078125e-07
ROPE_BASE = 10000.0

_CACHE = {}


def _to_bf16_bits(a):
    """Round fp32 ndarray to bf16 (ml_dtypes, matches mybir dt mapping)."""
    import ml_dtypes
    return np.ascontiguousarray(a, dtype=np.float32).astype(ml_dtypes.bfloat16)


def _round_f32r(a, bits=10):
    u = np.ascontiguousarray(a, dtype=np.float32).view(np.uint32).astype(np.uint64)
    u = (u + (1 << (bits - 1))) & ~np.uint64((1 << bits) - 1)
    return np.minimum(u, 0xFFFFFFFF).astype(np.uint32).view(np.float32)


def _build_consts():
    inv_freq = (1.0 / (ROPE_BASE ** (np.arange(0, D, 2, dtype=np.float32) / np.float32(D)))).astype(np.float32)
    pos = np.arange(T, dtype=np.float32)
    freqs = np.outer(pos, inv_freq).astype(np.float32)      # [T, 32]
    cosr = np.ascontiguousarray(np.tile(np.cos(freqs).T, (HPC, 1))).astype(np.float32)
    sinr = np.ascontiguousarray(np.tile(np.sin(freqs).T, (HPC, 1))).astype(np.float32)
    ind32 = np.zeros((P, HPC), dtype=np.float32)
    for p_ in range(P):
        ind32[p_, p_ // 32] = 1.0
    bc32 = np.zeros((HPC, P), dtype=np.float32)
    for p_ in range(P):
        bc32[p_ // 32, p_] = 1.0
    selpair = np.zeros((P, 2 * P), dtype=np.float32)
    for c in range(2):
        for m in range(P):
            selpair[32 * (2 * c + m // 64), 128 * c + m] = 1.0
    tri = np.tril(np.ones((P, P), np.float32)).T            # tri[p, c] = c >= p
    return dict(cosr=cosr, sinr=sinr,
                ind32=_round_f32r(ind32), bc32=_round_f32r(bc32),
                selpair=_round_f32r(selpair), tri=_to_bf16_bits(tri))


def _build_module():
    import concourse.bacc as bacc
    import concourse.mybir as mybir
    import concourse.tile as tile

    f32 = mybir.dt.float32
    f32r = mybir.dt.float32r
    bf16 = mybir.dt.bfloat16
    Exp = mybir.ActivationFunctionType.Exp
    Ln = mybir.ActivationFunctionType.Ln
    Alu = mybir.AluOpType

    nc = bacc.Bacc("TRN2", target_bir_lowering=False, debug=False,
                   num_devices=N_CORES)

    xt_d = nc.dram_tensor("xt", [C, T], f32r, kind="ExternalInput").ap()
    wq_d = nc.dram_tensor("wq", [C, 256], f32r, kind="ExternalInput").ap()
    wk_d = nc.dram_tensor("wk", [C, 256], f32r, kind="ExternalInput").ap()
    wv_d = nc.dram_tensor("wv", [C, 256], f32r, kind="ExternalInput").ap()
    wp_d = nc.dram_tensor("wp", [256, C], bf16, kind="ExternalInput").ap()
    cosr_d = nc.dram_tensor("cosr", [P, T], f32, kind="ExternalInput").ap()
    sinr_d = nc.dram_tensor("sinr", [P, T], f32, kind="ExternalInput").ap()
    ind32_d = nc.dram_tensor("ind32", [P, HPC], f32r, kind="ExternalInput").ap()
    bc32_d = nc.dram_tensor("bc32", [HPC, P], f32r, kind="ExternalInput").ap()
    selpair_d = nc.dram_tensor("selpair", [P, 2 * P], f32r, kind="ExternalInput").ap()
    tri_d = nc.dram_tensor("tri", [P, P], bf16, kind="ExternalInput").ap()
    out_d = nc.dram_tensor("outT", [C, T], bf16, kind="ExternalOutput").ap()
    if _KDBG:
        dbg_qT0 = nc.dram_tensor("dbg_qT0", [P, T], f32, kind="ExternalOutput").ap()
        dbg_kT0 = nc.dram_tensor("dbg_kT0", [P, T], f32, kind="ExternalOutput").ap()
        dbg_v0 = nc.dram_tensor("dbg_v0", [P, 256], bf16, kind="ExternalOutput").ap()
        dbg_inv0 = nc.dram_tensor("dbg_inv0", [P, TB], f32, kind="ExternalOutput").ap()
        dbg_yT0 = nc.dram_tensor("dbg_yT0", [P, T], bf16, kind="ExternalOutput").ap()
        dbg_e0 = nc.dram_tensor("dbg_e0", [P, 2 * TB], bf16, kind="ExternalOutput").ap()

    with tile.TileContext(nc) as tc:
        with (
            tc.tile_pool(name="sb", bufs=1) as sb,
            tc.tile_pool(name="tr", bufs=2) as tr,
            tc.tile_pool(name="ps", bufs=1, space="PSUM") as ps,
        ):
            # Pin the combined ln+exp table once; every Ln/Exp below is
            # servable from set 6 so no further table loads are inserted.
            ld = mybir.InstLoadActFuncSet(
                name=nc.get_next_instruction_name(), ins=[], outs=[],
                act_func_set_id=6)
            nc.scalar.add_instruction(ld)

            def direct_load(name, dram_slice, shape, dt=f32r):
                t_r = sb.tile(shape, dt, tag=name, name=name)
                nc.sync.dma_start(out=t_r[:], in_=dram_slice)
                return t_r

            ind32_r = direct_load("ind32r", ind32_d[:, :], [P, HPC])
            bc32_r = direct_load("bc32r", bc32_d[:, :], [HPC, P])
            selpair_r = direct_load("selpairr", selpair_d[:, :], [P, 2 * P])
            tri_r = direct_load("trir", tri_d[:, :], [P, P], bf16)
            cosr_t = direct_load("cosr", cosr_d[:, :], [P, T], f32)
            sinr_t = direct_load("sinr", sinr_d[:, :], [P, T], f32)
            wq_r = [direct_load(f"wqr{k}", wq_d[k * P:(k + 1) * P, :], [P, 256])
                    for k in range(KCH)]
            wk_r = [direct_load(f"wkr{k}", wk_d[k * P:(k + 1) * P, :], [P, 256])
                    for k in range(KCH)]
            wv_r = [direct_load(f"wvr{k}", wv_d[k * P:(k + 1) * P, :], [P, 256])
                    for k in range(KCH)]
            wp_r = [direct_load(f"wpr{c}", wp_d[c * P:(c + 1) * P, :], [P, C], bf16)
                    for c in range(2)]

            ones32 = sb.tile([P, 32], bf16, tag="ones32", name="ones32")
            nc.gpsimd.memset(ones32[:], 1.0)
            eps_t = sb.tile([HPC, 1], f32, tag="epst", name="eps_t")
            nc.gpsimd.memset(eps_t[:], RMS_EPS)

            # persistent activations
            qT = [sb.tile([P, T], f32r, tag=f"qT{c}", name=f"qT{c}")
                  for c in range(2)]
            kT = [sb.tile([P, T], f32r, tag=f"kT{c}", name=f"kT{c}")
                  for c in range(2)]
            v_r = [sb.tile([P, 256], bf16, tag=f"v{s}", name=f"v{s}")
                   for s in range(T // P)]
            yT = [sb.tile([P, T], bf16, tag=f"yT{c}", name=f"yT{c}")
                  for c in range(2)]

            # ================= Phase 1: projections =================
            for n in range(NB):
                nsl = slice(n * TB, (n + 1) * TB)
                xr_t = []
                for k in range(KCH):
                    xr = tr.tile([P, TB], f32r, tag="xr", name=f"xr{n}_{k}", bufs=10)
                    nc.sync.dma_start(out=xr[:], in_=xt_d[k * P:(k + 1) * P, nsl])
                    xr_t.append(xr)
                # q/k projection into 2-bank PSUM tiles (x1 | x2 halves)
                pq = ps.tile([P, 2 * TB], f32, tag="sc", name=f"pq{n}", bufs=2)
                pk = ps.tile([P, 2 * TB], f32, tag="sc", name=f"pk{n}", bufs=2)
                for k in range(KCH):
                    st, sp = (k == 0), (k == KCH - 1)
                    nc.tensor.matmul(pq[:, 0:TB], lhsT=wq_r[k][:, 0:128],
                                     rhs=xr_t[k][:], start=st, stop=sp)
                    nc.tensor.matmul(pq[:, TB:2 * TB], lhsT=wq_r[k][:, 128:256],
                                     rhs=xr_t[k][:], start=st, stop=sp)
                    nc.tensor.matmul(pk[:, 0:TB], lhsT=wk_r[k][:, 0:128],
                                     rhs=xr_t[k][:], start=st, stop=sp)
                    nc.tensor.matmul(pk[:, TB:2 * TB], lhsT=wk_r[k][:, 128:256],
                                     rhs=xr_t[k][:], start=st, stop=sp)
                x1q = tr.tile([P, TB], f32, tag="x1q", name=f"x1q{n}", bufs=1)
                x2q = tr.tile([P, TB], f32, tag="x2q", name=f"x2q{n}", bufs=1)
                x1k = tr.tile([P, TB], f32, tag="x1k", name=f"x1k{n}", bufs=1)
                x2k = tr.tile([P, TB], f32, tag="x2k", name=f"x2k{n}", bufs=1)
                nc.vector.tensor_copy(x1q[:], pq[:, 0:TB])
                nc.scalar.copy(x2q[:], pq[:, TB:2 * TB])
                nc.vector.tensor_copy(x1k[:], pk[:, 0:TB])
                nc.scalar.copy(x2k[:], pk[:, TB:2 * TB])
                # v projection (tokens on partitions): 2 PSUM tiles x 2 s_rel
                pv0 = ps.tile([P, TB], f32, tag="yv", name=f"pv0{n}", bufs=2)
                pv1 = ps.tile([P, TB], f32, tag="yv", name=f"pv1{n}", bufs=2)
                for k in range(KCH):
                    # free-dim-split regions sharing a bank at the same tile
                    # position: start=True clears has_written for the whole
                    # bank, so ONLY the first region may restart the group
                    # (HW-verified). Col-tiled partition-split regions (Y/den
                    # below) instead need per-region starts.
                    st, sp = (k == 0), (k == KCH - 1)
                    nc.tensor.matmul(pv0[:, 0:256], lhsT=xr_t[k][:, 0:128],
                                     rhs=wv_r[k][:], start=st, stop=False)
                    nc.tensor.matmul(pv0[:, 256:512], lhsT=xr_t[k][:, 128:256],
                                     rhs=wv_r[k][:], start=False, stop=sp)
                    nc.tensor.matmul(pv1[:, 0:256], lhsT=xr_t[k][:, 256:384],
                                     rhs=wv_r[k][:], start=st, stop=False)
                    nc.tensor.matmul(pv1[:, 256:512], lhsT=xr_t[k][:, 384:512],
                                     rhs=wv_r[k][:], start=False, stop=sp)
                for s_rel in range(4):
                    pv = pv0 if s_rel < 2 else pv1
                    csl = slice((s_rel % 2) * 256, (s_rel % 2) * 256 + 256)
                    nc.scalar.copy(v_r[4 * n + s_rel][:], pv[:, csl])
                # ---- RMS-norm + RoPE per tensor ----
                for (x1, x2, dstT, eng) in ((x1q, x2q, qT, "q"),
                                            (x1k, x2k, kT, "k")):
                    e = nc.vector if eng == "q" else nc.gpsimd
                    sq1 = tr.tile([P, TB], f32r, tag=f"sq1{eng}", name=f"sq1{eng}{n}", bufs=1)
                    sq2 = tr.tile([P, TB], f32r, tag=f"sq2{eng}", name=f"sq2{eng}{n}", bufs=1)
                    e.tensor_mul(sq1[:], x1[:], x1[:])
                    e.tensor_mul(sq2[:], x2[:], x2[:])
                    mtag = "den" if eng == "q" else "tail"
                    ms = ps.tile([HPC, TB], f32, tag=mtag, name=f"ms{eng}{n}", bufs=1)
                    nc.tensor.matmul(ms[:], lhsT=ind32_r[:], rhs=sq1[:],
                                     start=True, stop=False)
                    nc.tensor.matmul(ms[:], lhsT=ind32_r[:], rhs=sq2[:],
                                     start=False, stop=True)
                    invc = tr.tile([HPC, TB], f32r, tag=f"invc{eng}", name=f"invc{eng}{n}")
                    nc.scalar.activation(invc[:], ms[:], Ln,
                                         bias=eps_t[:], scale=1.0 / 64.0)
                    nc.scalar.activation(invc[:], invc[:], Exp, scale=-0.5)
                    bc = ps.tile([P, TB], f32, tag=mtag, name=f"bc{eng}{n}", bufs=1)
                    nc.tensor.matmul(bc[:], lhsT=bc32_r[:], rhs=invc[:],
                                     start=True, stop=True)
                    nc.vector.tensor_mul(x1[:], x1[:], bc[:])
                    nc.vector.tensor_mul(x2[:], x2[:], bc[:])
                    # rope
                    m_a = tr.tile([P, TB], f32, tag=f"tmpA{eng}", name=f"ma{eng}{n}", bufs=1)
                    m_b = tr.tile([P, TB], f32, tag=f"tmpB{eng}", name=f"mb{eng}{n}", bufs=1)
                    rc1 = tr.tile([P, TB], f32r, tag=f"roc1{eng}", name=f"rc1{eng}{n}", bufs=1)
                    rc2 = tr.tile([P, TB], f32r, tag=f"roc2{eng}", name=f"rc2{eng}{n}", bufs=1)
                    e.tensor_mul(m_a[:], x1[:], cosr_t[:, nsl])
                    e.tensor_mul(m_b[:], x2[:], sinr_t[:, nsl])
                    e.tensor_add(rc1[:], m_a[:], m_b[:])
                    m_c = tr.tile([P, TB], f32, tag=f"tmpA{eng}", name=f"mc{eng}{n}", bufs=1)
                    m_d = tr.tile([P, TB], f32, tag=f"tmpB{eng}", name=f"md{eng}{n}", bufs=1)
                    e.tensor_mul(m_c[:], x2[:], cosr_t[:, nsl])
                    e.tensor_mul(m_d[:], x1[:], sinr_t[:, nsl])
                    e.tensor_sub(rc2[:], m_c[:], m_d[:])
                    # repack: head h (rows 32h of rc1/rc2) ->
                    #   dstT[h//2] rows 64*(h%2) (rc1) / 64*(h%2)+32 (rc2)
                    for h in range(HPC):
                        dst = dstT[h // 2]
                        rb = 64 * (h % 2)
                        hs = slice(32 * h, 32 * h + 32)
                        nc.sync.dma_start(out=dst[rb:rb + 32, nsl], in_=rc1[hs, :])
                        nc.sync.dma_start(out=dst[rb + 32:rb + 64, nsl], in_=rc2[hs, :])

            if _KDBG:
                nc.sync.dma_start(out=dbg_qT0[:, :], in_=qT[0][:].bitcast(f32))
                nc.sync.dma_start(out=dbg_kT0[:, :], in_=kT[0][:].bitcast(f32))
                nc.sync.dma_start(out=dbg_v0[:, :], in_=v_r[0][:])

            # ================= Phase 2: attention =================
            for j in range(NB):
                jsl = slice(j * TB, (j + 1) * TB)
                n_k = 4 * j + 4
                Y = [ps.tile([P, TB], f32, tag="yv", name=f"Y{c}_{j}", bufs=2)
                     for c in range(2)]
                den32 = ps.tile([P, TB], f32, tag="den", name=f"den{j}", bufs=1)
                for k in range(n_k):
                    r = k - 4 * j
                    mtrim = min(128 * r, 256) if r > 0 else 0   # f32r N>=256
                    ytrim = 128 * r if r > 0 else 0             # bf16 side
                    ksl = slice(k * P, (k + 1) * P)
                    st, sp = (k == 0), (k == n_k - 1)
                    for c in range(2):
                        S = ps.tile([P, 2 * TB], f32, tag="sc",
                                    name=f"S{c}_{j}_{k}", bufs=2)
                        for e_i in range(2):
                            rsl = slice(64 * e_i, 64 * e_i + 64)
                            nc.tensor.matmul(
                                S[:, e_i * TB + mtrim:(e_i + 1) * TB],
                                lhsT=kT[c][rsl, ksl],
                                rhs=qT[c][rsl, j * TB + mtrim:(j + 1) * TB],
                                start=True, stop=True)
                        eb = tr.tile([P, 2 * TB], bf16, tag="eb",
                                     name=f"e{c}_{j}_{k}", bufs=3)
                        if ytrim == 0:
                            nc.scalar.activation(eb[:], S[:], Exp, scale=0.125)
                        else:
                            in3 = S[:].rearrange("p (s n) -> p s n", s=2)
                            out3 = eb[:].rearrange("p (s n) -> p s n", s=2)
                            nc.scalar.activation(out3[:, :, ytrim:],
                                                 in3[:, :, ytrim:],
                                                 Exp, scale=0.125)
                        if r >= 0:
                            # causal triangle on the diagonal 128-col strip
                            for e_i in range(2):
                                stc = e_i * TB + 128 * r
                                nc.vector.tensor_mul(eb[:, stc:stc + 128],
                                                     eb[:, stc:stc + 128],
                                                     tri_r[:])
                        if _KDBG and j == 0 and k == 0 and c == 0:
                            nc.sync.dma_start(out=dbg_e0[:, :], in_=eb[:])
                        for e_i in range(2):
                            h = 2 * c + e_i
                            # per-region groups (see pv comment)
                            nc.tensor.matmul(
                                Y[c][64 * e_i:64 * e_i + 64, ytrim:TB],
                                lhsT=v_r[k][:, 64 * h:64 * h + 64],
                                rhs=eb[:, e_i * TB + ytrim:(e_i + 1) * TB],
                                start=st, stop=sp,
                                tile_position=(0, 64 * e_i))
                            nc.tensor.matmul(
                                den32[32 * h:32 * h + 32, ytrim:TB],
                                lhsT=ones32[:],
                                rhs=eb[:, e_i * TB + ytrim:(e_i + 1) * TB],
                                start=st, stop=sp,
                                tile_position=(0, 32 * h))
                # ---- normalize + copy out to yT ----
                invf = tr.tile([P, TB], f32, tag="invf", name=f"invf{j}")
                nc.vector.reciprocal_approx_fast(invf[:], den32[:])
                if _KDBG and j == 0:
                    nc.sync.dma_start(out=dbg_inv0[:, :], in_=invf[:])
                invden = tr.tile([P, TB], f32r, tag="invden", name=f"invden{j}")
                nc.vector.tensor_copy(invden[:], invf[:])
                for c in range(2):
                    bcp = ps.tile([P, TB], f32, tag="tail", name=f"bcp{c}_{j}", bufs=1)
                    nc.tensor.matmul(bcp[:], lhsT=selpair_r[:, c * P:(c + 1) * P],
                                     rhs=invden[:], start=True, stop=True)
                    inv64 = tr.tile([P, TB], f32, tag="inv64", name=f"inv64{c}_{j}")
                    nc.vector.tensor_copy(inv64[:], bcp[:])
                    nc.vector.tensor_mul(yT[c][:, jsl], Y[c][:], inv64[:])
                # ---- out-projection for this row block ----
                for o in range(8):
                    osl = slice(o * P, (o + 1) * P)
                    po = ps.tile([P, TB], f32, tag="tail", name=f"po{o}_{j}", bufs=1)
                    nc.tensor.matmul(po[:], lhsT=wp_r[0][:, osl], rhs=yT[0][:, jsl],
                                     start=True, stop=False)
                    nc.tensor.matmul(po[:], lhsT=wp_r[1][:, osl], rhs=yT[1][:, jsl],
                                     start=False, stop=True)
                    ob = tr.tile([P, TB], bf16, tag="ob", name=f"ob{o}_{j}", bufs=3)
                    nc.vector.tensor_copy(ob[:], po[:])
                    nc.sync.dma_start(out=out_d[osl, jsl], in_=ob[:])
            if _KDBG:
                nc.sync.dma_start(out=dbg_yT0[:, :], in_=yT[0][:])

    nc.compile()
    return nc


def _get_module():
    if "nc" not in _CACHE:
        _CACHE["nc"] = _build_module()
        _CACHE["consts"] = _build_consts()
    return _CACHE["nc"], _CACHE["consts"]


def _core_inputs(x, w_q, w_k, w_v, w_proj, core):
    b = core // 4
    g = core % 4
    heads = [4 * g + j for j in range(HPC)]

    xt = _round_f32r(np.ascontiguousarray(x[b].T))        # [C, T]

    perm = np.empty(256, dtype=np.int64)
    for m in range(128):
        perm[m] = 64 * heads[m // 32] + (m % 32)             # x1 half
        perm[128 + m] = 64 * heads[m // 32] + 32 + (m % 32)  # x2 half
    wq = _round_f32r(np.ascontiguousarray(w_q[perm, :].T))   # [C, 256]
    wk = _round_f32r(np.ascontiguousarray(w_k[perm, :].T))

    vperm = np.empty(256, dtype=np.int64)
    for m in range(256):
        vperm[m] = 64 * heads[m // 64] + (m % 64)
    wv = _round_f32r(np.ascontiguousarray(w_v[vperm, :].T))  # [C, 256]
    wp = _to_bf16_bits(np.ascontiguousarray(w_proj[:, vperm].T))  # [256, C]

    return dict(xt=xt, wq=wq, wk=wk, wv=wv, wp=wp)


def kernel(x, w_q, w_k, w_v, w_proj, _trace=False, _trace_cores=None):
    from concourse.bass_utils import run_bass_kernel_spmd

    nc, consts = _get_module()
    x = np.asarray(x, dtype=np.float32)
    in_maps = []
    for core in range(N_CORES):
        m = _core_inputs(np.asarray(x), np.asarray(w_q), np.asarray(w_k),
                         np.asarray(w_v), np.asarray(w_proj), core)
        m.update(consts)
        in_maps.append(m)

    res = run_bass_kernel_spmd(nc, in_maps, list(range(N_CORES)),
                               trace=_trace, trace_cores=_trace_cores)
    outs = []
    for c in range(N_CORES):
        o = np.asarray(res.results[c]["outT"]).astype(np.float32)
        outs.append(o)
    out = np.empty((B, T, C), dtype=np.float32)
    for b in range(B):
        acc = outs[4 * b].astype(np.float32)
        for g in range(1, 4):
            acc = acc + outs[4 * b + g]
        out[b] = acc.T
    if _trace:
        kernel._last_exec_time_ns = res.exec_time_ns
        kernel._last_results = res
    return out


# revision 21
# speedup vs baseline: 1.1643x; 1.0801x over previous
"""Causal self-attention (RMSNorm-QK + RoPE) Trainium2 Bass kernel.

Problem: B=2, T=2048, C=1024, H=16 heads, D=64.
Sharding: 8 cores = 2 (batch) x 4 (head groups of 4 heads). Each core does
q/k/v projections for its 4 heads, attention, and a column-parallel partial
out-projection; host sums 4 bf16 partials per batch and transposes.

Key structure (vs. a naive port):
 - single activation-table set (natural_log_exp_and_others) preloaded: Ln,
   Exp and Square share one table -> zero ACT_TABLE_LOAD thrash.
 - scores matmuls 2-way row-tiled (K=64 per head, heads of a pair in rows
   0-63 / 64-127), attn*V and denominator matmuls col-tiled via
   tile_position -> PE array halves run concurrently.
 - softmax exp batched per head-pair [128, 1024] out of a 2-bank PSUM tile,
   trimmed to the causal range with a 3D access pattern.
 - denominator = col-tiled ones32 matmuls (32-row replicated), inverted with
   reciprocal_approx_fast on DVE, applied via a selpair matmul + a fused
   multiply on the Y PSUM->SBUF copy.
 - f32r on the q/k/scores path, bf16 on the v/probs/out path.
"""

import os
import sys

for _p in ("/opt/trn_rl_repo",):
    if _p not in sys.path:
        sys.path.append(_p)

import numpy as np

_KDBG = bool(os.environ.get("KDBG"))

B, T, C = 2, 2048, 1024
H_TOT, D = 16, 64
HPC = 4               # heads per core
N_CORES = 8
P = 128               # partitions
NB = 4                # t-blocks of 512
TB = 512              # t-block size
KCH = 8               # C / 128 contraction chunks
RMS_EPS = 1.1920928955078125e-07
ROPE_BASE = 10000.0

_CACHE = {}


def _to_bf16_bits(a):
    """Round fp32 ndarray to bf16 (ml_dtypes, matches mybir dt mapping)."""
    import ml_dtypes
    return np.ascontiguousarray(a, dtype=np.float32).astype(ml_dtypes.bfloat16)


def _round_f32r(a, bits=10):
    u = np.ascontiguousarray(a, dtype=np.float32).view(np.uint32).astype(np.uint64)
    u = (u + (1 << (bits - 1))) & ~np.uint64((1 << bits) - 1)
    return np.minimum(u, 0xFFFFFFFF).astype(np.uint32).view(np.float32)


def _build_consts():
    inv_freq = (1.0 / (ROPE_BASE ** (np.arange(0, D, 2, dtype=np.float32) / np.float32(D)))).astype(np.float32)
    pos = np.arange(T, dtype=np.float32)
    freqs = np.outer(pos, inv_freq).astype(np.float32)      # [T, 32]
    cosr = np.ascontiguousarray(np.tile(np.cos(freqs).T, (HPC, 1))).astype(np.float32)
    sinr = np.ascontiguousarray(np.tile(np.sin(freqs).T, (HPC, 1))).astype(np.float32)
    ind32 = np.zeros((P, HPC), dtype=np.float32)
    for p_ in range(P):
        ind32[p_, p_ // 32] = 1.0
    bc32 = np.zeros((HPC, P), dtype=np.float32)
    for p_ in range(P):
        bc32[p_ // 32, p_] = 1.0
    selpair = np.zeros((P, 2 * P), dtype=np.float32)
    for c in range(2):
        for m in range(P):
            selpair[32 * (2 * c + m // 64), 128 * c + m] = 1.0
    tri = np.tril(np.ones((P, P), np.float32)).T            # tri[p, c] = c >= p
    return dict(cosr=cosr, sinr=sinr,
                ind32=_round_f32r(ind32), bc32=_round_f32r(bc32),
                selpair=_round_f32r(selpair), tri=_to_bf16_bits(tri))


def _build_module():
    import concourse.bacc as bacc
    import concourse.mybir as mybir
    import concourse.tile as tile

    f32 = mybir.dt.float32
    f32r = mybir.dt.float32r
    bf16 = mybir.dt.bfloat16
    Exp = mybir.ActivationFunctionType.Exp
    Ln = mybir.ActivationFunctionType.Ln
    Alu = mybir.AluOpType

    nc = bacc.Bacc("TRN2", target_bir_lowering=False, debug=False,
                   num_devices=N_CORES)

    xt_d = nc.dram_tensor("xt", [C, T], f32r, kind="ExternalInput").ap()
    wq_d = nc.dram_tensor("wq", [C, 256], f32r, kind="ExternalInput").ap()
    wk_d = nc.dram_tensor("wk", [C, 256], f32r, kind="ExternalInput").ap()
    wv_d = nc.dram_tensor("wv", [C, 256], f32r, kind="ExternalInput").ap()
    wp_d = nc.dram_tensor("wp", [256, C], bf16, kind="ExternalInput").ap()
    cosr_d = nc.dram_tensor("cosr", [P, T], f32, kind="ExternalInput").ap()
    sinr_d = nc.dram_tensor("sinr", [P, T], f32, kind="ExternalInput").ap()
    ind32_d = nc.dram_tensor("ind32", [P, HPC], f32r, kind="ExternalInput").ap()
    bc32_d = nc.dram_tensor("bc32", [HPC, P], f32r, kind="ExternalInput").ap()
    selpair_d = nc.dram_tensor("selpair", [P, 2 * P], f32r, kind="ExternalInput").ap()
    tri_d = nc.dram_tensor("tri", [P, P], bf16, kind="ExternalInput").ap()
    out_d = nc.dram_tensor("outT", [C, T], bf16, kind="ExternalOutput").ap()
    if _KDBG:
        dbg_qT0 = nc.dram_tensor("dbg_qT0", [P, T], f32, kind="ExternalOutput").ap()
        dbg_kT0 = nc.dram_tensor("dbg_kT0", [P, T], f32, kind="ExternalOutput").ap()
        dbg_v0 = nc.dram_tensor("dbg_v0", [P, 256], bf16, kind="ExternalOutput").ap()
        dbg_inv0 = nc.dram_tensor("dbg_inv0", [P, TB], f32, kind="ExternalOutput").ap()
        dbg_yT0 = nc.dram_tensor("dbg_yT0", [P, T], bf16, kind="ExternalOutput").ap()
        dbg_e0 = nc.dram_tensor("dbg_e0", [P, 2 * TB], bf16, kind="ExternalOutput").ap()

    with tile.TileContext(nc) as tc:
        with (
            tc.tile_pool(name="sb", bufs=1) as sb,
            tc.tile_pool(name="tr", bufs=2) as tr,
            tc.tile_pool(name="ps", bufs=1, space="PSUM") as ps,
        ):
            # Pin the combined ln+exp table once; every Ln/Exp below is
            # servable from set 6 so no further table loads are inserted.
            ld = mybir.InstLoadActFuncSet(
                name=nc.get_next_instruction_name(), ins=[], outs=[],
                act_func_set_id=6)
            nc.scalar.add_instruction(ld)

            def direct_load(name, dram_slice, shape, dt=f32r):
                t_r = sb.tile(shape, dt, tag=name, name=name)
                nc.sync.dma_start(out=t_r[:], in_=dram_slice)
                return t_r

            # DMA order matters: the first projection matmuls need only
            # wq/wk (and the first xt chunks, loaded in the n-loop) — load
            # those first so compute starts ~5us in, and defer everything
            # only needed later (rope tables, wv, wp, attention consts).
            wq_r = [direct_load(f"wqr{k}", wq_d[k * P:(k + 1) * P, :], [P, 256])
                    for k in range(KCH)]
            wk_r = [direct_load(f"wkr{k}", wk_d[k * P:(k + 1) * P, :], [P, 256])
                    for k in range(KCH)]
            ind32_r = direct_load("ind32r", ind32_d[:, :], [P, HPC])
            bc32_r = direct_load("bc32r", bc32_d[:, :], [HPC, P])
            wv_r = [direct_load(f"wvr{k}", wv_d[k * P:(k + 1) * P, :], [P, 256])
                    for k in range(KCH)]
            cosr_t = direct_load("cosr", cosr_d[:, :], [P, T], f32)
            sinr_t = direct_load("sinr", sinr_d[:, :], [P, T], f32)
            selpair_r = direct_load("selpairr", selpair_d[:, :], [P, 2 * P])
            tri_r = direct_load("trir", tri_d[:, :], [P, P], bf16)
            wp_r = [direct_load(f"wpr{c}", wp_d[c * P:(c + 1) * P, :], [P, C], bf16)
                    for c in range(2)]

            ones32 = sb.tile([P, 32], bf16, tag="ones32", name="ones32")
            nc.gpsimd.memset(ones32[:], 1.0)
            eps_t = sb.tile([HPC, 1], f32, tag="epst", name="eps_t")
            nc.gpsimd.memset(eps_t[:], RMS_EPS)

            # persistent activations
            qT = [sb.tile([P, T], f32r, tag=f"qT{c}", name=f"qT{c}")
                  for c in range(2)]
            kT = [sb.tile([P, T], f32r, tag=f"kT{c}", name=f"kT{c}")
                  for c in range(2)]
            v_r = [sb.tile([P, 256], bf16, tag=f"v{s}", name=f"v{s}")
                   for s in range(T // P)]
            yT = [sb.tile([P, T], bf16, tag=f"yT{c}", name=f"yT{c}")
                  for c in range(2)]

            # ================= Phase 1: projections =================
            for n in range(NB):
                nsl = slice(n * TB, (n + 1) * TB)
                xr_t = []
                for k in range(KCH):
                    xr = tr.tile([P, TB], f32r, tag="xr", name=f"xr{n}_{k}", bufs=10)
                    nc.sync.dma_start(out=xr[:], in_=xt_d[k * P:(k + 1) * P, nsl])
                    xr_t.append(xr)
                # q/k projection into 2-bank PSUM tiles (x1 | x2 halves)
                pq = ps.tile([P, 2 * TB], f32, tag="sc", name=f"pq{n}", bufs=2)
                pk = ps.tile([P, 2 * TB], f32, tag="sc", name=f"pk{n}", bufs=2)
                for k in range(KCH):
                    st, sp = (k == 0), (k == KCH - 1)
                    nc.tensor.matmul(pq[:, 0:TB], lhsT=wq_r[k][:, 0:128],
                                     rhs=xr_t[k][:], start=st, stop=sp)
                    nc.tensor.matmul(pq[:, TB:2 * TB], lhsT=wq_r[k][:, 128:256],
                                     rhs=xr_t[k][:], start=st, stop=sp)
                    nc.tensor.matmul(pk[:, 0:TB], lhsT=wk_r[k][:, 0:128],
                                     rhs=xr_t[k][:], start=st, stop=sp)
                    nc.tensor.matmul(pk[:, TB:2 * TB], lhsT=wk_r[k][:, 128:256],
                                     rhs=xr_t[k][:], start=st, stop=sp)
                x1q = tr.tile([P, TB], f32, tag="x1q", name=f"x1q{n}", bufs=1)
                x2q = tr.tile([P, TB], f32, tag="x2q", name=f"x2q{n}", bufs=1)
                x1k = tr.tile([P, TB], f32, tag="x1k", name=f"x1k{n}", bufs=1)
                x2k = tr.tile([P, TB], f32, tag="x2k", name=f"x2k{n}", bufs=1)
                nc.vector.tensor_copy(x1q[:], pq[:, 0:TB])
                nc.scalar.copy(x2q[:], pq[:, TB:2 * TB])
                nc.vector.tensor_copy(x1k[:], pk[:, 0:TB])
                nc.scalar.copy(x2k[:], pk[:, TB:2 * TB])
                # v projection (tokens on partitions): 2 PSUM tiles x 2 s_rel
                pv0 = ps.tile([P, TB], f32, tag="yv", name=f"pv0{n}", bufs=2)
                pv1 = ps.tile([P, TB], f32, tag="yv", name=f"pv1{n}", bufs=2)
                for k in range(KCH):
                    # free-dim-split regions sharing a bank at the same tile
                    # position: start=True clears has_written for the whole
                    # bank, so ONLY the first region may restart the group
                    # (HW-verified). Col-tiled partition-split regions (Y/den
                    # below) instead need per-region starts.
                    st, sp = (k == 0), (k == KCH - 1)
                    nc.tensor.matmul(pv0[:, 0:256], lhsT=xr_t[k][:, 0:128],
                                     rhs=wv_r[k][:], start=st, stop=False)
                    nc.tensor.matmul(pv0[:, 256:512], lhsT=xr_t[k][:, 128:256],
                                     rhs=wv_r[k][:], start=False, stop=sp)
                    nc.tensor.matmul(pv1[:, 0:256], lhsT=xr_t[k][:, 256:384],
                                     rhs=wv_r[k][:], start=st, stop=False)
                    nc.tensor.matmul(pv1[:, 256:512], lhsT=xr_t[k][:, 384:512],
                                     rhs=wv_r[k][:], start=False, stop=sp)
                for s_rel in range(4):
                    pv = pv0 if s_rel < 2 else pv1
                    csl = slice((s_rel % 2) * 256, (s_rel % 2) * 256 + 256)
                    nc.scalar.copy(v_r[4 * n + s_rel][:], pv[:, csl])
                # ---- RMS-norm + RoPE, q/k chains interleaved so the PE's
                # ms/bc matmuls of both tensors queue back-to-back and the
                # Ln/Exp waits overlap with other engines' work ----
                tens = ((x1q, x2q, qT, "q", nc.vector),
                        (x1k, x2k, kT, "k", nc.gpsimd))
                sqs, mss, invcs, bcs = {}, {}, {}, {}
                for (x1, x2, dstT, eng, e) in tens:
                    sq1 = tr.tile([P, TB], f32r, tag=f"sq1{eng}", name=f"sq1{eng}{n}", bufs=1)
                    sq2 = tr.tile([P, TB], f32r, tag=f"sq2{eng}", name=f"sq2{eng}{n}", bufs=1)
                    e.tensor_mul(sq1[:], x1[:], x1[:])
                    e.tensor_mul(sq2[:], x2[:], x2[:])
                    sqs[eng] = (sq1, sq2)
                for (x1, x2, dstT, eng, e) in tens:
                    mtag = "den" if eng == "q" else "tail"
                    ms = ps.tile([HPC, TB], f32, tag=mtag, name=f"ms{eng}{n}", bufs=1)
                    sq1, sq2 = sqs[eng]
                    nc.tensor.matmul(ms[:], lhsT=ind32_r[:], rhs=sq1[:],
                                     start=True, stop=False)
                    nc.tensor.matmul(ms[:], lhsT=ind32_r[:], rhs=sq2[:],
                                     start=False, stop=True)
                    mss[eng] = ms
                for (x1, x2, dstT, eng, e) in tens:
                    invc = tr.tile([HPC, TB], f32r, tag=f"invc{eng}", name=f"invc{eng}{n}")
                    nc.scalar.activation(invc[:], mss[eng][:], Ln,
                                         bias=eps_t[:], scale=1.0 / 64.0)
                    nc.scalar.activation(invc[:], invc[:], Exp, scale=-0.5)
                    invcs[eng] = invc
                for (x1, x2, dstT, eng, e) in tens:
                    mtag = "den" if eng == "q" else "tail"
                    bc = ps.tile([P, TB], f32, tag=mtag, name=f"bc{eng}{n}", bufs=1)
                    nc.tensor.matmul(bc[:], lhsT=bc32_r[:], rhs=invcs[eng][:],
                                     start=True, stop=True)
                    bcs[eng] = bc
                for (x1, x2, dstT, eng, e) in tens:
                    bc = bcs[eng]
                    nc.vector.tensor_mul(x1[:], x1[:], bc[:])
                    nc.vector.tensor_mul(x2[:], x2[:], bc[:])
                    # rope
                    m_a = tr.tile([P, TB], f32, tag=f"tmpA{eng}", name=f"ma{eng}{n}", bufs=1)
                    m_b = tr.tile([P, TB], f32, tag=f"tmpB{eng}", name=f"mb{eng}{n}", bufs=1)
                    rc1 = tr.tile([P, TB], f32r, tag=f"roc1{eng}", name=f"rc1{eng}{n}", bufs=1)
                    rc2 = tr.tile([P, TB], f32r, tag=f"roc2{eng}", name=f"rc2{eng}{n}", bufs=1)
                    e.tensor_mul(m_a[:], x1[:], cosr_t[:, nsl])
                    e.tensor_mul(m_b[:], x2[:], sinr_t[:, nsl])
                    e.tensor_add(rc1[:], m_a[:], m_b[:])
                    m_c = tr.tile([P, TB], f32, tag=f"tmpA{eng}", name=f"mc{eng}{n}", bufs=1)
                    m_d = tr.tile([P, TB], f32, tag=f"tmpB{eng}", name=f"md{eng}{n}", bufs=1)
                    e.tensor_mul(m_c[:], x2[:], cosr_t[:, nsl])
                    e.tensor_mul(m_d[:], x1[:], sinr_t[:, nsl])
                    e.tensor_sub(rc2[:], m_c[:], m_d[:])
                    # repack: head h (rows 32h of rc1/rc2) ->
                    #   dstT[h//2] rows 64*(h%2) (rc1) / 64*(h%2)+32 (rc2)
                    for h in range(HPC):
                        dst = dstT[h // 2]
                        rb = 64 * (h % 2)
                        hs = slice(32 * h, 32 * h + 32)
                        nc.sync.dma_start(out=dst[rb:rb + 32, nsl], in_=rc1[hs, :])
                        nc.sync.dma_start(out=dst[rb + 32:rb + 64, nsl], in_=rc2[hs, :])

            if _KDBG:
                nc.sync.dma_start(out=dbg_qT0[:, :], in_=qT[0][:].bitcast(f32))
                nc.sync.dma_start(out=dbg_kT0[:, :], in_=kT[0][:].bitcast(f32))
                nc.sync.dma_start(out=dbg_v0[:, :], in_=v_r[0][:])

            # ================= Phase 2: attention =================
            for j in range(NB):
                jsl = slice(j * TB, (j + 1) * TB)
                n_k = 4 * j + 4
                Y = [ps.tile([P, TB], f32, tag="yv", name=f"Y{c}_{j}", bufs=2)
                     for c in range(2)]
                den32 = ps.tile([P, TB], f32, tag="den", name=f"den{j}", bufs=1)
                def emit_scores(k):
                    r = k - 4 * j
                    mtrim = min(128 * r, 256) if r > 0 else 0   # f32r N>=256
                    ksl = slice(k * P, (k + 1) * P)
                    Ss = []
                    for c in range(2):
                        S = ps.tile([P, 2 * TB], f32, tag="sc",
                                    name=f"S{c}_{j}_{k}", bufs=2)
                        for e_i in range(2):
                            rsl = slice(64 * e_i, 64 * e_i + 64)
                            nc.tensor.matmul(
                                S[:, e_i * TB + mtrim:(e_i + 1) * TB],
                                lhsT=kT[c][rsl, ksl],
                                rhs=qT[c][rsl, j * TB + mtrim:(j + 1) * TB],
                                start=True, stop=True)
                        Ss.append(S)
                    return Ss

                # software pipeline: scores(k+1) are emitted BEFORE the
                # Y/den matmuls of block k, so the in-order PE stream works
                # on next-block scores while ScalarE runs this block's exp.
                Ss = emit_scores(0)
                for k in range(n_k):
                    r = k - 4 * j
                    ytrim = 128 * r if r > 0 else 0             # bf16 side
                    st, sp = (k == 0), (k == n_k - 1)
                    ebs = []
                    for c in range(2):
                        S = Ss[c]
                        eb = tr.tile([P, 2 * TB], bf16, tag="eb",
                                     name=f"e{c}_{j}_{k}", bufs=3)
                        if ytrim == 0:
                            nc.scalar.activation(eb[:], S[:], Exp, scale=0.125)
                        else:
                            in3 = S[:].rearrange("p (s n) -> p s n", s=2)
                            out3 = eb[:].rearrange("p (s n) -> p s n", s=2)
                            nc.scalar.activation(out3[:, :, ytrim:],
                                                 in3[:, :, ytrim:],
                                                 Exp, scale=0.125)
                        if r >= 0:
                            # causal triangle on the diagonal 128-col strip
                            # (GpSimd: idle during attention, off DVE)
                            for e_i in range(2):
                                stc = e_i * TB + 128 * r
                                nc.gpsimd.tensor_mul(eb[:, stc:stc + 128],
                                                     eb[:, stc:stc + 128],
                                                     tri_r[:])
                        if _KDBG and j == 0 and k == 0 and c == 0:
                            nc.sync.dma_start(out=dbg_e0[:, :], in_=eb[:])
                        ebs.append(eb)
                    if k + 1 < n_k:
                        Ss = emit_scores(k + 1)
                    for c in range(2):
                        eb = ebs[c]
                        for e_i in range(2):
                            h = 2 * c + e_i
                            # per-region accumulation groups (col-tiled)
                            nc.tensor.matmul(
                                Y[c][64 * e_i:64 * e_i + 64, ytrim:TB],
                                lhsT=v_r[k][:, 64 * h:64 * h + 64],
                                rhs=eb[:, e_i * TB + ytrim:(e_i + 1) * TB],
                                start=st, stop=sp,
                                tile_position=(0, 64 * e_i))
                            nc.tensor.matmul(
                                den32[32 * h:32 * h + 32, ytrim:TB],
                                lhsT=ones32[:],
                                rhs=eb[:, e_i * TB + ytrim:(e_i + 1) * TB],
                                start=st, stop=sp,
                                tile_position=(0, 32 * h))
                # ---- normalize + copy out to yT ----
                invf = tr.tile([P, TB], f32, tag="invf", name=f"invf{j}")
                nc.vector.reciprocal_approx_fast(invf[:], den32[:])
                if _KDBG and j == 0:
                    nc.sync.dma_start(out=dbg_inv0[:, :], in_=invf[:])
                invden = tr.tile([P, TB], f32r, tag="invden", name=f"invden{j}")
                nc.vector.tensor_copy(invden[:], invf[:])
                for c in range(2):
                    bcp = ps.tile([P, TB], f32, tag="tail", name=f"bcp{c}_{j}", bufs=1)
                    nc.tensor.matmul(bcp[:], lhsT=selpair_r[:, c * P:(c + 1) * P],
                                     rhs=invden[:], start=True, stop=True)
                    inv64 = tr.tile([P, TB], f32, tag="inv64", name=f"inv64{c}_{j}")
                    nc.vector.tensor_copy(inv64[:], bcp[:])
                    nc.vector.tensor_mul(yT[c][:, jsl], Y[c][:], inv64[:])
                # ---- out-projection for this row block ----
                for o in range(8):
                    osl = slice(o * P, (o + 1) * P)
                    po = ps.tile([P, TB], f32, tag="tail", name=f"po{o}_{j}", bufs=1)
                    nc.tensor.matmul(po[:], lhsT=wp_r[0][:, osl], rhs=yT[0][:, jsl],
                                     start=True, stop=False)
                    nc.tensor.matmul(po[:], lhsT=wp_r[1][:, osl], rhs=yT[1][:, jsl],
                                     start=False, stop=True)
                    ob = tr.tile([P, TB], bf16, tag="ob", name=f"ob{o}_{j}", bufs=3)
                    nc.vector.tensor_copy(ob[:], po[:])
                    nc.sync.dma_start(out=out_d[osl, jsl], in_=ob[:])
            if _KDBG:
                nc.sync.dma_start(out=dbg_yT0[:, :], in_=yT[0][:])

    nc.compile()
    return nc


def _get_module():
    if "nc" not in _CACHE:
        _CACHE["nc"] = _build_module()
        _CACHE["consts"] = _build_consts()
    return _CACHE["nc"], _CACHE["consts"]


def _core_inputs(x, w_q, w_k, w_v, w_proj, core):
    b = core // 4
    g = core % 4
    heads = [4 * g + j for j in range(HPC)]

    xt = _round_f32r(np.ascontiguousarray(x[b].T))        # [C, T]

    perm = np.empty(256, dtype=np.int64)
    for m in range(128):
        perm[m] = 64 * heads[m // 32] + (m % 32)             # x1 half
        perm[128 + m] = 64 * heads[m // 32] + 32 + (m % 32)  # x2 half
    wq = _round_f32r(np.ascontiguousarray(w_q[perm, :].T))   # [C, 256]
    wk = _round_f32r(np.ascontiguousarray(w_k[perm, :].T))

    vperm = np.empty(256, dtype=np.int64)
    for m in range(256):
        vperm[m] = 64 * heads[m // 64] + (m % 64)
    wv = _round_f32r(np.ascontiguousarray(w_v[vperm, :].T))  # [C, 256]
    wp = _to_bf16_bits(np.ascontiguousarray(w_proj[:, vperm].T))  # [256, C]

    return dict(xt=xt, wq=wq, wk=wk, wv=wv, wp=wp)


def kernel(x, w_q, w_k, w_v, w_proj, _trace=False, _trace_cores=None):
    from concourse.bass_utils import run_bass_kernel_spmd

    nc, consts = _get_module()
    x = np.asarray(x, dtype=np.float32)
    in_maps = []
    for core in range(N_CORES):
        m = _core_inputs(np.asarray(x), np.asarray(w_q), np.asarray(w_k),
                         np.asarray(w_v), np.asarray(w_proj), core)
        m.update(consts)
        in_maps.append(m)

    res = run_bass_kernel_spmd(nc, in_maps, list(range(N_CORES)),
                               trace=_trace, trace_cores=_trace_cores)
    outs = []
    for c in range(N_CORES):
        o = np.asarray(res.results[c]["outT"]).astype(np.float32)
        outs.append(o)
    out = np.empty((B, T, C), dtype=np.float32)
    for b in range(B):
        acc = outs[4 * b].astype(np.float32)
        for g in range(1, 4):
            acc = acc + outs[4 * b + g]
        out[b] = acc.T
    if _trace:
        kernel._last_exec_time_ns = res.exec_time_ns
        kernel._last_results = res
    return out


# revision 27
# speedup vs baseline: 1.1703x; 1.0051x over previous
"""Causal self-attention (RMSNorm-QK + RoPE) Trainium2 Bass kernel.

Problem: B=2, T=2048, C=1024, H=16 heads, D=64.
Sharding: 8 cores = 2 (batch) x 4 (head groups of 4 heads). Each core does
q/k/v projections for its 4 heads, attention, and a column-parallel partial
out-projection; host sums 4 bf16 partials per batch and transposes.

Key structure (vs. a naive port):
 - single activation-table set (natural_log_exp_and_others) preloaded: Ln,
   Exp and Square share one table -> zero ACT_TABLE_LOAD thrash.
 - scores matmuls 2-way row-tiled (K=64 per head, heads of a pair in rows
   0-63 / 64-127), attn*V and denominator matmuls col-tiled via
   tile_position -> PE array halves run concurrently.
 - softmax exp batched per head-pair [128, 1024] out of a 2-bank PSUM tile,
   trimmed to the causal range with a 3D access pattern.
 - denominator = col-tiled ones32 matmuls (32-row replicated), inverted with
   reciprocal_approx_fast on DVE, applied via a selpair matmul + a fused
   multiply on the Y PSUM->SBUF copy.
 - f32r on the q/k/scores path, bf16 on the v/probs/out path.
"""

import os
import sys

for _p in ("/opt/trn_rl_repo",):
    if _p not in sys.path:
        sys.path.append(_p)

import numpy as np

_KDBG = bool(os.environ.get("KDBG"))

B, T, C = 2, 2048, 1024
H_TOT, D = 16, 64
HPC = 4               # heads per core
N_CORES = 8
P = 128               # partitions
NB = 4                # t-blocks of 512
TB = 512              # t-block size
KCH = 8               # C / 128 contraction chunks
RMS_EPS = 1.1920928955078125e-07
ROPE_BASE = 10000.0

_CACHE = {}


def _to_bf16_bits(a):
    """Round fp32 ndarray to bf16 (ml_dtypes, matches mybir dt mapping)."""
    import ml_dtypes
    return np.ascontiguousarray(a, dtype=np.float32).astype(ml_dtypes.bfloat16)


def _round_f32r(a, bits=10):
    u = np.ascontiguousarray(a, dtype=np.float32).view(np.uint32).astype(np.uint64)
    u = (u + (1 << (bits - 1))) & ~np.uint64((1 << bits) - 1)
    return np.minimum(u, 0xFFFFFFFF).astype(np.uint32).view(np.float32)


def _build_consts():
    inv_freq = (1.0 / (ROPE_BASE ** (np.arange(0, D, 2, dtype=np.float32) / np.float32(D)))).astype(np.float32)
    pos = np.arange(T, dtype=np.float32)
    freqs = np.outer(pos, inv_freq).astype(np.float32)      # [T, 32]
    cosr = np.ascontiguousarray(np.tile(np.cos(freqs).T, (HPC, 1))).astype(np.float32)
    sinr = np.ascontiguousarray(np.tile(np.sin(freqs).T, (HPC, 1))).astype(np.float32)
    ind32 = np.zeros((P, HPC), dtype=np.float32)
    for p_ in range(P):
        ind32[p_, p_ // 32] = 1.0
    bc32 = np.zeros((HPC, P), dtype=np.float32)
    for p_ in range(P):
        bc32[p_ // 32, p_] = 1.0
    selpair = np.zeros((P, 2 * P), dtype=np.float32)
    for c in range(2):
        for m in range(P):
            selpair[32 * (2 * c + m // 64), 128 * c + m] = 1.0
    tri = np.tril(np.ones((P, P), np.float32)).T            # tri[p, c] = c >= p
    return dict(cosr=cosr, sinr=sinr,
                ind32=_round_f32r(ind32), bc32=_round_f32r(bc32),
                selpair=_round_f32r(selpair), tri=_to_bf16_bits(tri))


def _build_module():
    import concourse.bacc as bacc
    import concourse.mybir as mybir
    import concourse.tile as tile

    f32 = mybir.dt.float32
    f32r = mybir.dt.float32r
    bf16 = mybir.dt.bfloat16
    Exp = mybir.ActivationFunctionType.Exp
    Ln = mybir.ActivationFunctionType.Ln
    Alu = mybir.AluOpType

    nc = bacc.Bacc("TRN2", target_bir_lowering=False, debug=False,
                   num_devices=N_CORES)

    xt_d = nc.dram_tensor("xt", [C, T], f32r, kind="ExternalInput").ap()
    wq_d = nc.dram_tensor("wq", [C, 256], f32r, kind="ExternalInput").ap()
    wk_d = nc.dram_tensor("wk", [C, 256], f32r, kind="ExternalInput").ap()
    wv_d = nc.dram_tensor("wv", [C, 256], f32r, kind="ExternalInput").ap()
    wp_d = nc.dram_tensor("wp", [256, C], bf16, kind="ExternalInput").ap()
    cosr_d = nc.dram_tensor("cosr", [P, T], f32, kind="ExternalInput").ap()
    sinr_d = nc.dram_tensor("sinr", [P, T], f32, kind="ExternalInput").ap()
    ind32_d = nc.dram_tensor("ind32", [P, HPC], f32r, kind="ExternalInput").ap()
    bc32_d = nc.dram_tensor("bc32", [HPC, P], f32r, kind="ExternalInput").ap()
    selpair_d = nc.dram_tensor("selpair", [P, 2 * P], f32r, kind="ExternalInput").ap()
    tri_d = nc.dram_tensor("tri", [P, P], bf16, kind="ExternalInput").ap()
    out_d = nc.dram_tensor("outT", [C, T], bf16, kind="ExternalOutput").ap()
    if _KDBG:
        dbg_qT0 = nc.dram_tensor("dbg_qT0", [P, T], f32, kind="ExternalOutput").ap()
        dbg_kT0 = nc.dram_tensor("dbg_kT0", [P, T], f32, kind="ExternalOutput").ap()
        dbg_v0 = nc.dram_tensor("dbg_v0", [P, 256], bf16, kind="ExternalOutput").ap()
        dbg_inv0 = nc.dram_tensor("dbg_inv0", [P, TB], f32, kind="ExternalOutput").ap()
        dbg_yT0 = nc.dram_tensor("dbg_yT0", [P, T], bf16, kind="ExternalOutput").ap()
        dbg_e0 = nc.dram_tensor("dbg_e0", [P, 2 * TB], bf16, kind="ExternalOutput").ap()

    with tile.TileContext(nc) as tc:
        with (
            tc.tile_pool(name="sb", bufs=1) as sb,
            tc.tile_pool(name="tr", bufs=2) as tr,
            tc.tile_pool(name="ps", bufs=1, space="PSUM") as ps,
        ):
            # Pin the combined ln+exp table once; every Ln/Exp below is
            # servable from set 6 so no further table loads are inserted.
            ld = mybir.InstLoadActFuncSet(
                name=nc.get_next_instruction_name(), ins=[], outs=[],
                act_func_set_id=6)
            nc.scalar.add_instruction(ld)

            def direct_load(name, dram_slice, shape, dt=f32r):
                t_r = sb.tile(shape, dt, tag=name, name=name)
                nc.sync.dma_start(out=t_r[:], in_=dram_slice)
                return t_r

            # DMA order matters: the first projection matmuls need only
            # wq/wk (and the first xt chunks, loaded in the n-loop) — load
            # those first so compute starts ~5us in, and defer everything
            # only needed later (rope tables, wv, wp, attention consts).
            wq_r = [direct_load(f"wqr{k}", wq_d[k * P:(k + 1) * P, :], [P, 256])
                    for k in range(KCH)]
            wk_r = [direct_load(f"wkr{k}", wk_d[k * P:(k + 1) * P, :], [P, 256])
                    for k in range(KCH)]
            ind32_r = direct_load("ind32r", ind32_d[:, :], [P, HPC])
            bc32_r = direct_load("bc32r", bc32_d[:, :], [HPC, P])
            # remaining loads are emitted lazily inside the n=0 body so
            # their transfers overlap the first projection matmuls
            wv_r = [sb.tile([P, 256], f32r, tag=f"wvr{k}", name=f"wvr{k}")
                    for k in range(KCH)]
            cosr_t = sb.tile([P, T], f32, tag="cosr", name="cosr")
            sinr_t = sb.tile([P, T], f32, tag="sinr", name="sinr")
            selpair_r = sb.tile([P, 2 * P], f32r, tag="selpairr", name="selpairr")
            tri_r = sb.tile([P, P], bf16, tag="trir", name="trir")
            wp_r = [sb.tile([P, C], bf16, tag=f"wpr{c}", name=f"wpr{c}")
                    for c in range(2)]

            def deferred_loads():
                for k in range(KCH):
                    nc.sync.dma_start(out=wv_r[k][:], in_=wv_d[k * P:(k + 1) * P, :])
                nc.sync.dma_start(out=cosr_t[:], in_=cosr_d[:, :])
                nc.sync.dma_start(out=sinr_t[:], in_=sinr_d[:, :])
                nc.scalar.dma_start(out=selpair_r[:], in_=selpair_d[:, :])
                nc.scalar.dma_start(out=tri_r[:], in_=tri_d[:, :])
                for c in range(2):
                    nc.scalar.dma_start(out=wp_r[c][:], in_=wp_d[c * P:(c + 1) * P, :])

            ones32 = sb.tile([P, 32], bf16, tag="ones32", name="ones32")
            nc.gpsimd.memset(ones32[:], 1.0)
            eps_t = sb.tile([HPC, 1], f32, tag="epst", name="eps_t")
            nc.gpsimd.memset(eps_t[:], RMS_EPS)

            # persistent activations
            qT = [sb.tile([P, T], f32r, tag=f"qT{c}", name=f"qT{c}")
                  for c in range(2)]
            kT = [sb.tile([P, T], f32r, tag=f"kT{c}", name=f"kT{c}")
                  for c in range(2)]
            v_r = [sb.tile([P, 256], bf16, tag=f"v{s}", name=f"v{s}")
                   for s in range(T // P)]
            yT = [sb.tile([P, T], bf16, tag=f"yT{c}", name=f"yT{c}")
                  for c in range(2)]

            # ================= Phase 1: projections =================
            for n in range(NB):
                nsl = slice(n * TB, (n + 1) * TB)
                xr_t = []
                for k in range(KCH):
                    xr = tr.tile([P, TB], f32r, tag="xr", name=f"xr{n}_{k}", bufs=10)
                    nc.sync.dma_start(out=xr[:], in_=xt_d[k * P:(k + 1) * P, nsl])
                    xr_t.append(xr)
                # q/k projection into 2-bank PSUM tiles (x1 | x2 halves)
                pq = ps.tile([P, 2 * TB], f32, tag="sc", name=f"pq{n}", bufs=2)
                pk = ps.tile([P, 2 * TB], f32, tag="sc", name=f"pk{n}", bufs=2)
                for k in range(KCH):
                    st, sp = (k == 0), (k == KCH - 1)
                    nc.tensor.matmul(pq[:, 0:TB], lhsT=wq_r[k][:, 0:128],
                                     rhs=xr_t[k][:], start=st, stop=sp)
                    nc.tensor.matmul(pq[:, TB:2 * TB], lhsT=wq_r[k][:, 128:256],
                                     rhs=xr_t[k][:], start=st, stop=sp)
                    nc.tensor.matmul(pk[:, 0:TB], lhsT=wk_r[k][:, 0:128],
                                     rhs=xr_t[k][:], start=st, stop=sp)
                    nc.tensor.matmul(pk[:, TB:2 * TB], lhsT=wk_r[k][:, 128:256],
                                     rhs=xr_t[k][:], start=st, stop=sp)
                if n == 0:
                    deferred_loads()
                x1q = tr.tile([P, TB], f32, tag="x1q", name=f"x1q{n}", bufs=1)
                x2q = tr.tile([P, TB], f32, tag="x2q", name=f"x2q{n}", bufs=1)
                x1k = tr.tile([P, TB], f32, tag="x1k", name=f"x1k{n}", bufs=1)
                x2k = tr.tile([P, TB], f32, tag="x2k", name=f"x2k{n}", bufs=1)
                nc.vector.tensor_copy(x1q[:], pq[:, 0:TB])
                nc.scalar.copy(x2q[:], pq[:, TB:2 * TB])
                nc.vector.tensor_copy(x1k[:], pk[:, 0:TB])
                nc.scalar.copy(x2k[:], pk[:, TB:2 * TB])
                # v projection (tokens on partitions): 2 PSUM tiles x 2 s_rel
                pv0 = ps.tile([P, TB], f32, tag="yv", name=f"pv0{n}", bufs=2)
                pv1 = ps.tile([P, TB], f32, tag="yv", name=f"pv1{n}", bufs=2)
                for k in range(KCH):
                    # free-dim-split regions sharing a bank at the same tile
                    # position: start=True clears has_written for the whole
                    # bank, so ONLY the first region may restart the group
                    # (HW-verified). Col-tiled partition-split regions (Y/den
                    # below) instead need per-region starts.
                    st, sp = (k == 0), (k == KCH - 1)
                    nc.tensor.matmul(pv0[:, 0:256], lhsT=xr_t[k][:, 0:128],
                                     rhs=wv_r[k][:], start=st, stop=False)
                    nc.tensor.matmul(pv0[:, 256:512], lhsT=xr_t[k][:, 128:256],
                                     rhs=wv_r[k][:], start=False, stop=sp)
                    nc.tensor.matmul(pv1[:, 0:256], lhsT=xr_t[k][:, 256:384],
                                     rhs=wv_r[k][:], start=st, stop=False)
                    nc.tensor.matmul(pv1[:, 256:512], lhsT=xr_t[k][:, 384:512],
                                     rhs=wv_r[k][:], start=False, stop=sp)
                for s_rel in range(4):
                    pv = pv0 if s_rel < 2 else pv1
                    csl = slice((s_rel % 2) * 256, (s_rel % 2) * 256 + 256)
                    nc.scalar.copy(v_r[4 * n + s_rel][:], pv[:, csl])
                # ---- RMS-norm + RoPE, q/k chains interleaved so the PE's
                # ms/bc matmuls of both tensors queue back-to-back and the
                # Ln/Exp waits overlap with other engines' work ----
                tens = ((x1q, x2q, qT, "q", nc.vector),
                        (x1k, x2k, kT, "k", nc.gpsimd))
                sqs, mss, invcs, bcs = {}, {}, {}, {}
                for (x1, x2, dstT, eng, e) in tens:
                    sq1 = tr.tile([P, TB], f32r, tag=f"sq1{eng}", name=f"sq1{eng}{n}", bufs=1)
                    sq2 = tr.tile([P, TB], f32r, tag=f"sq2{eng}", name=f"sq2{eng}{n}", bufs=1)
                    e.tensor_mul(sq1[:], x1[:], x1[:])
                    e.tensor_mul(sq2[:], x2[:], x2[:])
                    sqs[eng] = (sq1, sq2)
                for (x1, x2, dstT, eng, e) in tens:
                    mtag = "den" if eng == "q" else "tail"
                    ms = ps.tile([HPC, TB], f32, tag=mtag, name=f"ms{eng}{n}", bufs=1)
                    sq1, sq2 = sqs[eng]
                    nc.tensor.matmul(ms[:], lhsT=ind32_r[:], rhs=sq1[:],
                                     start=True, stop=False)
                    nc.tensor.matmul(ms[:], lhsT=ind32_r[:], rhs=sq2[:],
                                     start=False, stop=True)
                    mss[eng] = ms
                for (x1, x2, dstT, eng, e) in tens:
                    invc = tr.tile([HPC, TB], f32r, tag=f"invc{eng}", name=f"invc{eng}{n}")
                    nc.scalar.activation(invc[:], mss[eng][:], Ln,
                                         bias=eps_t[:], scale=1.0 / 64.0)
                    nc.scalar.activation(invc[:], invc[:], Exp, scale=-0.5)
                    invcs[eng] = invc
                for (x1, x2, dstT, eng, e) in tens:
                    mtag = "den" if eng == "q" else "tail"
                    bc = ps.tile([P, TB], f32, tag=mtag, name=f"bc{eng}{n}", bufs=1)
                    nc.tensor.matmul(bc[:], lhsT=bc32_r[:], rhs=invcs[eng][:],
                                     start=True, stop=True)
                    bcs[eng] = bc
                for (x1, x2, dstT, eng, e) in tens:
                    bc = bcs[eng]
                    nc.vector.tensor_mul(x1[:], x1[:], bc[:])
                    nc.vector.tensor_mul(x2[:], x2[:], bc[:])
                    # rope
                    m_a = tr.tile([P, TB], f32, tag=f"tmpA{eng}", name=f"ma{eng}{n}", bufs=1)
                    m_b = tr.tile([P, TB], f32, tag=f"tmpB{eng}", name=f"mb{eng}{n}", bufs=1)
                    rc1 = tr.tile([P, TB], f32r, tag=f"roc1{eng}", name=f"rc1{eng}{n}", bufs=1)
                    rc2 = tr.tile([P, TB], f32r, tag=f"roc2{eng}", name=f"rc2{eng}{n}", bufs=1)
                    e.tensor_mul(m_a[:], x1[:], cosr_t[:, nsl])
                    e.tensor_mul(m_b[:], x2[:], sinr_t[:, nsl])
                    e.tensor_add(rc1[:], m_a[:], m_b[:])
                    m_c = tr.tile([P, TB], f32, tag=f"tmpA{eng}", name=f"mc{eng}{n}", bufs=1)
                    m_d = tr.tile([P, TB], f32, tag=f"tmpB{eng}", name=f"md{eng}{n}", bufs=1)
                    e.tensor_mul(m_c[:], x2[:], cosr_t[:, nsl])
                    e.tensor_mul(m_d[:], x1[:], sinr_t[:, nsl])
                    e.tensor_sub(rc2[:], m_c[:], m_d[:])
                    # repack: head h (rows 32h of rc1/rc2) ->
                    #   dstT[h//2] rows 64*(h%2) (rc1) / 64*(h%2)+32 (rc2)
                    for h in range(HPC):
                        dst = dstT[h // 2]
                        rb = 64 * (h % 2)
                        hs = slice(32 * h, 32 * h + 32)
                        nc.scalar.dma_start(out=dst[rb:rb + 32, nsl], in_=rc1[hs, :])
                        nc.scalar.dma_start(out=dst[rb + 32:rb + 64, nsl], in_=rc2[hs, :])

            if _KDBG:
                nc.sync.dma_start(out=dbg_qT0[:, :], in_=qT[0][:].bitcast(f32))
                nc.sync.dma_start(out=dbg_kT0[:, :], in_=kT[0][:].bitcast(f32))
                nc.sync.dma_start(out=dbg_v0[:, :], in_=v_r[0][:])

            # ================= Phase 2: attention =================
            # out-projection units of row j are sprinkled into row j+1's
            # k-loop so their PSUM-bank round-trips hide under attention
            pending_out = []

            def emit_outproj(jo, o):
                josl = slice(jo * TB, (jo + 1) * TB)
                osl = slice(o * P, (o + 1) * P)
                po = ps.tile([P, TB], f32, tag="tail", name=f"po{o}_{jo}", bufs=1)
                nc.tensor.matmul(po[:], lhsT=wp_r[0][:, osl], rhs=yT[0][:, josl],
                                 start=True, stop=False)
                nc.tensor.matmul(po[:], lhsT=wp_r[1][:, osl], rhs=yT[1][:, josl],
                                 start=False, stop=True)
                ob = tr.tile([P, TB], bf16, tag="ob", name=f"ob{o}_{jo}", bufs=3)
                nc.vector.tensor_copy(ob[:], po[:])
                nc.scalar.dma_start(out=out_d[osl, josl], in_=ob[:])

            for j in range(NB):
                jsl = slice(j * TB, (j + 1) * TB)
                n_k = 4 * j + 4
                Y = [ps.tile([P, TB], f32, tag="yv", name=f"Y{c}_{j}", bufs=2)
                     for c in range(2)]
                den32 = ps.tile([P, TB], f32, tag="den", name=f"den{j}", bufs=1)
                def emit_scores(k):
                    r = k - 4 * j
                    mtrim = min(128 * r, 256) if r > 0 else 0   # f32r N>=256
                    ksl = slice(k * P, (k + 1) * P)
                    Ss = []
                    for c in range(2):
                        S = ps.tile([P, 2 * TB], f32, tag="sc",
                                    name=f"S{c}_{j}_{k}", bufs=2)
                        for e_i in range(2):
                            rsl = slice(64 * e_i, 64 * e_i + 64)
                            nc.tensor.matmul(
                                S[:, e_i * TB + mtrim:(e_i + 1) * TB],
                                lhsT=kT[c][rsl, ksl],
                                rhs=qT[c][rsl, j * TB + mtrim:(j + 1) * TB],
                                start=True, stop=True)
                        Ss.append(S)
                    return Ss

                # software pipeline: scores(k+1) are emitted BEFORE the
                # Y/den matmuls of block k, so the in-order PE stream works
                # on next-block scores while ScalarE runs this block's exp.
                Ss = emit_scores(0)
                for k in range(n_k):
                    r = k - 4 * j
                    ytrim = 128 * r if r > 0 else 0             # bf16 side
                    st, sp = (k == 0), (k == n_k - 1)
                    ebs = []
                    for c in range(2):
                        S = Ss[c]
                        eb = tr.tile([P, 2 * TB], bf16, tag="eb",
                                     name=f"e{c}_{j}_{k}", bufs=3)
                        if ytrim == 0:
                            nc.scalar.activation(eb[:], S[:], Exp, scale=0.125)
                        else:
                            in3 = S[:].rearrange("p (s n) -> p s n", s=2)
                            out3 = eb[:].rearrange("p (s n) -> p s n", s=2)
                            nc.scalar.activation(out3[:, :, ytrim:],
                                                 in3[:, :, ytrim:],
                                                 Exp, scale=0.125)
                        if r >= 0:
                            # causal triangle on the diagonal 128-col strip
                            # (GpSimd: idle during attention, off DVE)
                            for e_i in range(2):
                                stc = e_i * TB + 128 * r
                                nc.gpsimd.tensor_mul(eb[:, stc:stc + 128],
                                                     eb[:, stc:stc + 128],
                                                     tri_r[:])
                        if _KDBG and j == 0 and k == 0 and c == 0:
                            nc.sync.dma_start(out=dbg_e0[:, :], in_=eb[:])
                        ebs.append(eb)
                    if k + 1 < n_k:
                        Ss = emit_scores(k + 1)
                    if pending_out:
                        emit_outproj(*pending_out.pop(0))
                    for c in range(2):
                        eb = ebs[c]
                        for e_i in range(2):
                            h = 2 * c + e_i
                            # per-region accumulation groups (col-tiled)
                            nc.tensor.matmul(
                                Y[c][64 * e_i:64 * e_i + 64, ytrim:TB],
                                lhsT=v_r[k][:, 64 * h:64 * h + 64],
                                rhs=eb[:, e_i * TB + ytrim:(e_i + 1) * TB],
                                start=st, stop=sp,
                                tile_position=(0, 64 * e_i))
                            nc.tensor.matmul(
                                den32[32 * h:32 * h + 32, ytrim:TB],
                                lhsT=ones32[:],
                                rhs=eb[:, e_i * TB + ytrim:(e_i + 1) * TB],
                                start=st, stop=sp,
                                tile_position=(0, 32 * h))
                # ---- normalize + copy out to yT ----
                invf = tr.tile([P, TB], f32, tag="invf", name=f"invf{j}")
                nc.vector.reciprocal_approx_fast(invf[:], den32[:])
                if _KDBG and j == 0:
                    nc.sync.dma_start(out=dbg_inv0[:, :], in_=invf[:])
                invden = tr.tile([P, TB], f32r, tag="invden", name=f"invden{j}")
                nc.vector.tensor_copy(invden[:], invf[:])
                for c in range(2):
                    bcp = ps.tile([P, TB], f32, tag="tail", name=f"bcp{c}_{j}", bufs=1)
                    nc.tensor.matmul(bcp[:], lhsT=selpair_r[:, c * P:(c + 1) * P],
                                     rhs=invden[:], start=True, stop=True)
                    inv64 = tr.tile([P, TB], f32, tag="inv64", name=f"inv64{c}_{j}")
                    nc.vector.tensor_copy(inv64[:], bcp[:])
                    nc.vector.tensor_mul(yT[c][:, jsl], Y[c][:], inv64[:])
                # queue this row's out-projection for the next row's loop
                pending_out.extend((j, o) for o in range(8))
            while pending_out:
                emit_outproj(*pending_out.pop(0))
            if _KDBG:
                nc.sync.dma_start(out=dbg_yT0[:, :], in_=yT[0][:])

    nc.compile()
    return nc


def _get_module():
    if "nc" not in _CACHE:
        _CACHE["nc"] = _build_module()
        _CACHE["consts"] = _build_consts()
    return _CACHE["nc"], _CACHE["consts"]


def _core_inputs(x, w_q, w_k, w_v, w_proj, core):
    b = core // 4
    g = core % 4
    heads = [4 * g + j for j in range(HPC)]

    xt = _round_f32r(np.ascontiguousarray(x[b].T))        # [C, T]

    perm = np.empty(256, dtype=np.int64)
    for m in range(128):
        perm[m] = 64 * heads[m // 32] + (m % 32)             # x1 half
        perm[128 + m] = 64 * heads[m // 32] + 32 + (m % 32)  # x2 half
    wq = _round_f32r(np.ascontiguousarray(w_q[perm, :].T))   # [C, 256]
    wk = _round_f32r(np.ascontiguousarray(w_k[perm, :].T))

    vperm = np.empty(256, dtype=np.int64)
    for m in range(256):
        vperm[m] = 64 * heads[m // 64] + (m % 64)
    wv = _round_f32r(np.ascontiguousarray(w_v[vperm, :].T))  # [C, 256]
    wp = _to_bf16_bits(np.ascontiguousarray(w_proj[:, vperm].T))  # [256, C]

    return dict(xt=xt, wq=wq, wk=wk, wv=wv, wp=wp)


def kernel(x, w_q, w_k, w_v, w_proj, _trace=False, _trace_cores=None):
    from concourse.bass_utils import run_bass_kernel_spmd

    nc, consts = _get_module()
    x = np.asarray(x, dtype=np.float32)
    in_maps = []
    for core in range(N_CORES):
        m = _core_inputs(np.asarray(x), np.asarray(w_q), np.asarray(w_k),
                         np.asarray(w_v), np.asarray(w_proj), core)
        m.update(consts)
        in_maps.append(m)

    res = run_bass_kernel_spmd(nc, in_maps, list(range(N_CORES)),
                               trace=_trace, trace_cores=_trace_cores)
    outs = []
    for c in range(N_CORES):
        o = np.asarray(res.results[c]["outT"]).astype(np.float32)
        outs.append(o)
    out = np.empty((B, T, C), dtype=np.float32)
    for b in range(B):
        acc = outs[4 * b].astype(np.float32)
        for g in range(1, 4):
            acc = acc + outs[4 * b + g]
        out[b] = acc.T
    if _trace:
        kernel._last_exec_time_ns = res.exec_time_ns
        kernel._last_results = res
    return out


# revision 28
# speedup vs baseline: 1.2131x; 1.0366x over previous
"""Causal self-attention (RMSNorm-QK + RoPE) Trainium2 Bass kernel.

Problem: B=2, T=2048, C=1024, H=16 heads, D=64.
Sharding: 8 cores = 2 (batch) x 4 (head groups of 4 heads). Each core does
q/k/v projections for its 4 heads, attention, and a column-parallel partial
out-projection; host sums 4 bf16 partials per batch and transposes.

Key structure (vs. a naive port):
 - single activation-table set (natural_log_exp_and_others) preloaded: Ln,
   Exp and Square share one table -> zero ACT_TABLE_LOAD thrash.
 - scores matmuls 2-way row-tiled (K=64 per head, heads of a pair in rows
   0-63 / 64-127), attn*V and denominator matmuls col-tiled via
   tile_position -> PE array halves run concurrently.
 - softmax exp batched per head-pair [128, 1024] out of a 2-bank PSUM tile,
   trimmed to the causal range with a 3D access pattern.
 - denominator = col-tiled ones32 matmuls (32-row replicated), inverted with
   reciprocal_approx_fast on DVE, applied via a selpair matmul + a fused
   multiply on the Y PSUM->SBUF copy.
 - f32r on the q/k/scores path, bf16 on the v/probs/out path.
"""

import os
import sys

for _p in ("/opt/trn_rl_repo",):
    if _p not in sys.path:
        sys.path.append(_p)

import numpy as np

_KDBG = bool(os.environ.get("KDBG"))

B, T, C = 2, 2048, 1024
H_TOT, D = 16, 64
HPC = 4               # heads per core
N_CORES = 8
P = 128               # partitions
NB = 4                # t-blocks of 512
TB = 512              # t-block size
KCH = 8               # C / 128 contraction chunks
RMS_EPS = 1.1920928955078125e-07
ROPE_BASE = 10000.0

_CACHE = {}


def _to_bf16_bits(a):
    """Round fp32 ndarray to bf16 (ml_dtypes, matches mybir dt mapping)."""
    import ml_dtypes
    return np.ascontiguousarray(a, dtype=np.float32).astype(ml_dtypes.bfloat16)


def _round_f32r(a, bits=10):
    u = np.ascontiguousarray(a, dtype=np.float32).view(np.uint32).astype(np.uint64)
    u = (u + (1 << (bits - 1))) & ~np.uint64((1 << bits) - 1)
    return np.minimum(u, 0xFFFFFFFF).astype(np.uint32).view(np.float32)


def _build_consts():
    inv_freq = (1.0 / (ROPE_BASE ** (np.arange(0, D, 2, dtype=np.float32) / np.float32(D)))).astype(np.float32)
    pos = np.arange(T, dtype=np.float32)
    freqs = np.outer(pos, inv_freq).astype(np.float32)      # [T, 32]
    cosr = np.ascontiguousarray(np.tile(np.cos(freqs).T, (HPC, 1))).astype(np.float32)
    sinr = np.ascontiguousarray(np.tile(np.sin(freqs).T, (HPC, 1))).astype(np.float32)
    ind32 = np.zeros((P, HPC), dtype=np.float32)
    for p_ in range(P):
        ind32[p_, p_ // 32] = 1.0
    bc32 = np.zeros((HPC, P), dtype=np.float32)
    for p_ in range(P):
        bc32[p_ // 32, p_] = 1.0
    selpair = np.zeros((P, 2 * P), dtype=np.float32)
    for c in range(2):
        for m in range(P):
            selpair[32 * (2 * c + m // 64), 128 * c + m] = 1.0
    tri = np.tril(np.ones((P, P), np.float32)).T            # tri[p, c] = c >= p
    return dict(cosr=cosr, sinr=sinr,
                ind32=_round_f32r(ind32), bc32=_round_f32r(bc32),
                selpair=_round_f32r(selpair), tri=_to_bf16_bits(tri))


def _build_module():
    import concourse.bacc as bacc
    import concourse.mybir as mybir
    import concourse.tile as tile

    f32 = mybir.dt.float32
    f32r = mybir.dt.float32r
    bf16 = mybir.dt.bfloat16
    Exp = mybir.ActivationFunctionType.Exp
    Ln = mybir.ActivationFunctionType.Ln
    Alu = mybir.AluOpType

    nc = bacc.Bacc("TRN2", target_bir_lowering=False, debug=False,
                   num_devices=N_CORES)

    xt_d = nc.dram_tensor("xt", [C, T], bf16, kind="ExternalInput").ap()
    wq_d = nc.dram_tensor("wq", [C, 256], bf16, kind="ExternalInput").ap()
    wk_d = nc.dram_tensor("wk", [C, 256], bf16, kind="ExternalInput").ap()
    wv_d = nc.dram_tensor("wv", [C, 256], bf16, kind="ExternalInput").ap()
    wp_d = nc.dram_tensor("wp", [256, C], bf16, kind="ExternalInput").ap()
    cosr_d = nc.dram_tensor("cosr", [P, T], f32, kind="ExternalInput").ap()
    sinr_d = nc.dram_tensor("sinr", [P, T], f32, kind="ExternalInput").ap()
    ind32_d = nc.dram_tensor("ind32", [P, HPC], f32r, kind="ExternalInput").ap()
    bc32_d = nc.dram_tensor("bc32", [HPC, P], f32r, kind="ExternalInput").ap()
    selpair_d = nc.dram_tensor("selpair", [P, 2 * P], f32r, kind="ExternalInput").ap()
    tri_d = nc.dram_tensor("tri", [P, P], bf16, kind="ExternalInput").ap()
    out_d = nc.dram_tensor("outT", [C, T], bf16, kind="ExternalOutput").ap()
    if _KDBG:
        dbg_qT0 = nc.dram_tensor("dbg_qT0", [P, T], f32, kind="ExternalOutput").ap()
        dbg_kT0 = nc.dram_tensor("dbg_kT0", [P, T], f32, kind="ExternalOutput").ap()
        dbg_v0 = nc.dram_tensor("dbg_v0", [P, 256], bf16, kind="ExternalOutput").ap()
        dbg_inv0 = nc.dram_tensor("dbg_inv0", [P, TB], f32, kind="ExternalOutput").ap()
        dbg_yT0 = nc.dram_tensor("dbg_yT0", [P, T], bf16, kind="ExternalOutput").ap()
        dbg_e0 = nc.dram_tensor("dbg_e0", [P, 2 * TB], bf16, kind="ExternalOutput").ap()

    with tile.TileContext(nc) as tc:
        with (
            tc.tile_pool(name="sb", bufs=1) as sb,
            tc.tile_pool(name="tr", bufs=2) as tr,
            tc.tile_pool(name="ps", bufs=1, space="PSUM") as ps,
        ):
            # Pin the combined ln+exp table once; every Ln/Exp below is
            # servable from set 6 so no further table loads are inserted.
            ld = mybir.InstLoadActFuncSet(
                name=nc.get_next_instruction_name(), ins=[], outs=[],
                act_func_set_id=6)
            nc.scalar.add_instruction(ld)

            def direct_load(name, dram_slice, shape, dt=f32r):
                t_r = sb.tile(shape, dt, tag=name, name=name)
                nc.sync.dma_start(out=t_r[:], in_=dram_slice)
                return t_r

            # DMA order matters: the first projection matmuls need only
            # wq/wk (and the first xt chunks, loaded in the n-loop) — load
            # those first so compute starts ~5us in, and defer everything
            # only needed later (rope tables, wv, wp, attention consts).
            wq_r = [direct_load(f"wqr{k}", wq_d[k * P:(k + 1) * P, :], [P, 256], bf16)
                    for k in range(KCH)]
            wk_r = [direct_load(f"wkr{k}", wk_d[k * P:(k + 1) * P, :], [P, 256], bf16)
                    for k in range(KCH)]
            ind32_r = direct_load("ind32r", ind32_d[:, :], [P, HPC])
            bc32_r = direct_load("bc32r", bc32_d[:, :], [HPC, P])
            # remaining loads are emitted lazily inside the n=0 body so
            # their transfers overlap the first projection matmuls
            wv_r = [sb.tile([P, 256], bf16, tag=f"wvr{k}", name=f"wvr{k}")
                    for k in range(KCH)]
            cosr_t = sb.tile([P, T], f32, tag="cosr", name="cosr")
            sinr_t = sb.tile([P, T], f32, tag="sinr", name="sinr")
            selpair_r = sb.tile([P, 2 * P], f32r, tag="selpairr", name="selpairr")
            tri_r = sb.tile([P, P], bf16, tag="trir", name="trir")
            wp_r = [sb.tile([P, C], bf16, tag=f"wpr{c}", name=f"wpr{c}")
                    for c in range(2)]

            def deferred_loads():
                for k in range(KCH):
                    nc.sync.dma_start(out=wv_r[k][:], in_=wv_d[k * P:(k + 1) * P, :])
                nc.sync.dma_start(out=cosr_t[:], in_=cosr_d[:, :])
                nc.sync.dma_start(out=sinr_t[:], in_=sinr_d[:, :])
                nc.scalar.dma_start(out=selpair_r[:], in_=selpair_d[:, :])
                nc.scalar.dma_start(out=tri_r[:], in_=tri_d[:, :])
                for c in range(2):
                    nc.scalar.dma_start(out=wp_r[c][:], in_=wp_d[c * P:(c + 1) * P, :])

            ones32 = sb.tile([P, 32], bf16, tag="ones32", name="ones32")
            nc.gpsimd.memset(ones32[:], 1.0)
            eps_t = sb.tile([HPC, 1], f32, tag="epst", name="eps_t")
            nc.gpsimd.memset(eps_t[:], RMS_EPS)

            # persistent activations
            qT = [sb.tile([P, T], bf16, tag=f"qT{c}", name=f"qT{c}")
                  for c in range(2)]
            kT = [sb.tile([P, T], bf16, tag=f"kT{c}", name=f"kT{c}")
                  for c in range(2)]
            v_r = [sb.tile([P, 256], bf16, tag=f"v{s}", name=f"v{s}")
                   for s in range(T // P)]
            yT = [sb.tile([P, T], bf16, tag=f"yT{c}", name=f"yT{c}")
                  for c in range(2)]

            # ================= Phase 1: projections =================
            for n in range(NB):
                nsl = slice(n * TB, (n + 1) * TB)
                xr_t = []
                for k in range(KCH):
                    xr = tr.tile([P, TB], bf16, tag="xr", name=f"xr{n}_{k}", bufs=10)
                    nc.sync.dma_start(out=xr[:], in_=xt_d[k * P:(k + 1) * P, nsl])
                    xr_t.append(xr)
                # q/k projection into 2-bank PSUM tiles (x1 | x2 halves)
                pq = ps.tile([P, 2 * TB], f32, tag="sc", name=f"pq{n}", bufs=2)
                pk = ps.tile([P, 2 * TB], f32, tag="sc", name=f"pk{n}", bufs=2)
                for k in range(KCH):
                    st, sp = (k == 0), (k == KCH - 1)
                    nc.tensor.matmul(pq[:, 0:TB], lhsT=wq_r[k][:, 0:128],
                                     rhs=xr_t[k][:], start=st, stop=sp)
                    nc.tensor.matmul(pq[:, TB:2 * TB], lhsT=wq_r[k][:, 128:256],
                                     rhs=xr_t[k][:], start=st, stop=sp)
                    nc.tensor.matmul(pk[:, 0:TB], lhsT=wk_r[k][:, 0:128],
                                     rhs=xr_t[k][:], start=st, stop=sp)
                    nc.tensor.matmul(pk[:, TB:2 * TB], lhsT=wk_r[k][:, 128:256],
                                     rhs=xr_t[k][:], start=st, stop=sp)
                if n == 0:
                    deferred_loads()
                x1q = tr.tile([P, TB], f32, tag="x1q", name=f"x1q{n}", bufs=1)
                x2q = tr.tile([P, TB], f32, tag="x2q", name=f"x2q{n}", bufs=1)
                x1k = tr.tile([P, TB], f32, tag="x1k", name=f"x1k{n}", bufs=1)
                x2k = tr.tile([P, TB], f32, tag="x2k", name=f"x2k{n}", bufs=1)
                nc.vector.tensor_copy(x1q[:], pq[:, 0:TB])
                nc.scalar.copy(x2q[:], pq[:, TB:2 * TB])
                nc.vector.tensor_copy(x1k[:], pk[:, 0:TB])
                nc.scalar.copy(x2k[:], pk[:, TB:2 * TB])
                # v projection (tokens on partitions): 2 PSUM tiles x 2 s_rel
                pv0 = ps.tile([P, TB], f32, tag="yv", name=f"pv0{n}", bufs=2)
                pv1 = ps.tile([P, TB], f32, tag="yv", name=f"pv1{n}", bufs=2)
                for k in range(KCH):
                    # free-dim-split regions sharing a bank at the same tile
                    # position: start=True clears has_written for the whole
                    # bank, so ONLY the first region may restart the group
                    # (HW-verified). Col-tiled partition-split regions (Y/den
                    # below) instead need per-region starts.
                    st, sp = (k == 0), (k == KCH - 1)
                    nc.tensor.matmul(pv0[:, 0:256], lhsT=xr_t[k][:, 0:128],
                                     rhs=wv_r[k][:], start=st, stop=False)
                    nc.tensor.matmul(pv0[:, 256:512], lhsT=xr_t[k][:, 128:256],
                                     rhs=wv_r[k][:], start=False, stop=sp)
                    nc.tensor.matmul(pv1[:, 0:256], lhsT=xr_t[k][:, 256:384],
                                     rhs=wv_r[k][:], start=st, stop=False)
                    nc.tensor.matmul(pv1[:, 256:512], lhsT=xr_t[k][:, 384:512],
                                     rhs=wv_r[k][:], start=False, stop=sp)
                for s_rel in range(4):
                    pv = pv0 if s_rel < 2 else pv1
                    csl = slice((s_rel % 2) * 256, (s_rel % 2) * 256 + 256)
                    nc.scalar.copy(v_r[4 * n + s_rel][:], pv[:, csl])
                # ---- RMS-norm + RoPE, q/k chains interleaved so the PE's
                # ms/bc matmuls of both tensors queue back-to-back and the
                # Ln/Exp waits overlap with other engines' work ----
                tens = ((x1q, x2q, qT, "q", nc.vector),
                        (x1k, x2k, kT, "k", nc.gpsimd))
                sqs, mss, invcs, bcs = {}, {}, {}, {}
                for (x1, x2, dstT, eng, e) in tens:
                    sq1 = tr.tile([P, TB], f32r, tag=f"sq1{eng}", name=f"sq1{eng}{n}", bufs=1)
                    sq2 = tr.tile([P, TB], f32r, tag=f"sq2{eng}", name=f"sq2{eng}{n}", bufs=1)
                    e.tensor_mul(sq1[:], x1[:], x1[:])
                    e.tensor_mul(sq2[:], x2[:], x2[:])
                    sqs[eng] = (sq1, sq2)
                for (x1, x2, dstT, eng, e) in tens:
                    mtag = "den" if eng == "q" else "tail"
                    ms = ps.tile([HPC, TB], f32, tag=mtag, name=f"ms{eng}{n}", bufs=1)
                    sq1, sq2 = sqs[eng]
                    nc.tensor.matmul(ms[:], lhsT=ind32_r[:], rhs=sq1[:],
                                     start=True, stop=False)
                    nc.tensor.matmul(ms[:], lhsT=ind32_r[:], rhs=sq2[:],
                                     start=False, stop=True)
                    mss[eng] = ms
                for (x1, x2, dstT, eng, e) in tens:
                    invc = tr.tile([HPC, TB], f32r, tag=f"invc{eng}", name=f"invc{eng}{n}")
                    nc.scalar.activation(invc[:], mss[eng][:], Ln,
                                         bias=eps_t[:], scale=1.0 / 64.0)
                    nc.scalar.activation(invc[:], invc[:], Exp, scale=-0.5)
                    invcs[eng] = invc
                for (x1, x2, dstT, eng, e) in tens:
                    mtag = "den" if eng == "q" else "tail"
                    bc = ps.tile([P, TB], f32, tag=mtag, name=f"bc{eng}{n}", bufs=1)
                    nc.tensor.matmul(bc[:], lhsT=bc32_r[:], rhs=invcs[eng][:],
                                     start=True, stop=True)
                    bcs[eng] = bc
                for (x1, x2, dstT, eng, e) in tens:
                    bc = bcs[eng]
                    nc.vector.tensor_mul(x1[:], x1[:], bc[:])
                    nc.vector.tensor_mul(x2[:], x2[:], bc[:])
                    # rope
                    m_a = tr.tile([P, TB], f32, tag=f"tmpA{eng}", name=f"ma{eng}{n}", bufs=1)
                    m_b = tr.tile([P, TB], f32, tag=f"tmpB{eng}", name=f"mb{eng}{n}", bufs=1)
                    rc1 = tr.tile([P, TB], bf16, tag=f"roc1{eng}", name=f"rc1{eng}{n}", bufs=1)
                    rc2 = tr.tile([P, TB], bf16, tag=f"roc2{eng}", name=f"rc2{eng}{n}", bufs=1)
                    e.tensor_mul(m_a[:], x1[:], cosr_t[:, nsl])
                    e.tensor_mul(m_b[:], x2[:], sinr_t[:, nsl])
                    e.tensor_add(rc1[:], m_a[:], m_b[:])
                    m_c = tr.tile([P, TB], f32, tag=f"tmpA{eng}", name=f"mc{eng}{n}", bufs=1)
                    m_d = tr.tile([P, TB], f32, tag=f"tmpB{eng}", name=f"md{eng}{n}", bufs=1)
                    e.tensor_mul(m_c[:], x2[:], cosr_t[:, nsl])
                    e.tensor_mul(m_d[:], x1[:], sinr_t[:, nsl])
                    e.tensor_sub(rc2[:], m_c[:], m_d[:])
                    # repack: head h (rows 32h of rc1/rc2) ->
                    #   dstT[h//2] rows 64*(h%2) (rc1) / 64*(h%2)+32 (rc2)
                    for h in range(HPC):
                        dst = dstT[h // 2]
                        rb = 64 * (h % 2)
                        hs = slice(32 * h, 32 * h + 32)
                        nc.scalar.dma_start(out=dst[rb:rb + 32, nsl], in_=rc1[hs, :])
                        nc.scalar.dma_start(out=dst[rb + 32:rb + 64, nsl], in_=rc2[hs, :])

            if _KDBG:
                pass  # dbg_qT0 dump disabled (bf16)
                pass
                nc.sync.dma_start(out=dbg_v0[:, :], in_=v_r[0][:])

            # ================= Phase 2: attention =================
            # out-projection units of row j are sprinkled into row j+1's
            # k-loop so their PSUM-bank round-trips hide under attention
            pending_out = []

            def emit_outproj(jo, o):
                josl = slice(jo * TB, (jo + 1) * TB)
                osl = slice(o * P, (o + 1) * P)
                po = ps.tile([P, TB], f32, tag="tail", name=f"po{o}_{jo}", bufs=1)
                nc.tensor.matmul(po[:], lhsT=wp_r[0][:, osl], rhs=yT[0][:, josl],
                                 start=True, stop=False)
                nc.tensor.matmul(po[:], lhsT=wp_r[1][:, osl], rhs=yT[1][:, josl],
                                 start=False, stop=True)
                ob = tr.tile([P, TB], bf16, tag="ob", name=f"ob{o}_{jo}", bufs=3)
                nc.vector.tensor_copy(ob[:], po[:])
                nc.scalar.dma_start(out=out_d[osl, josl], in_=ob[:])

            for j in range(NB):
                jsl = slice(j * TB, (j + 1) * TB)
                n_k = 4 * j + 4
                Y = [ps.tile([P, TB], f32, tag="yv", name=f"Y{c}_{j}", bufs=2)
                     for c in range(2)]
                den32 = ps.tile([P, TB], f32, tag="den", name=f"den{j}", bufs=1)
                def emit_scores(k):
                    r = k - 4 * j
                    mtrim = 128 * r if r > 0 else 0   # bf16: trim fully
                    ksl = slice(k * P, (k + 1) * P)
                    Ss = []
                    for c in range(2):
                        S = ps.tile([P, 2 * TB], f32, tag="sc",
                                    name=f"S{c}_{j}_{k}", bufs=2)
                        for e_i in range(2):
                            rsl = slice(64 * e_i, 64 * e_i + 64)
                            nc.tensor.matmul(
                                S[:, e_i * TB + mtrim:(e_i + 1) * TB],
                                lhsT=kT[c][rsl, ksl],
                                rhs=qT[c][rsl, j * TB + mtrim:(j + 1) * TB],
                                start=True, stop=True)
                        Ss.append(S)
                    return Ss

                # software pipeline: scores(k+1) are emitted BEFORE the
                # Y/den matmuls of block k, so the in-order PE stream works
                # on next-block scores while ScalarE runs this block's exp.
                Ss = emit_scores(0)
                for k in range(n_k):
                    r = k - 4 * j
                    ytrim = 128 * r if r > 0 else 0             # bf16 side
                    st, sp = (k == 0), (k == n_k - 1)
                    ebs = []
                    for c in range(2):
                        S = Ss[c]
                        eb = tr.tile([P, 2 * TB], bf16, tag="eb",
                                     name=f"e{c}_{j}_{k}", bufs=3)
                        if ytrim == 0:
                            nc.scalar.activation(eb[:], S[:], Exp, scale=0.125)
                        else:
                            in3 = S[:].rearrange("p (s n) -> p s n", s=2)
                            out3 = eb[:].rearrange("p (s n) -> p s n", s=2)
                            nc.scalar.activation(out3[:, :, ytrim:],
                                                 in3[:, :, ytrim:],
                                                 Exp, scale=0.125)
                        if r >= 0:
                            # causal triangle on the diagonal 128-col strip
                            # (GpSimd: idle during attention, off DVE)
                            for e_i in range(2):
                                stc = e_i * TB + 128 * r
                                nc.gpsimd.tensor_mul(eb[:, stc:stc + 128],
                                                     eb[:, stc:stc + 128],
                                                     tri_r[:])
                        if _KDBG and j == 0 and k == 0 and c == 0:
                            nc.sync.dma_start(out=dbg_e0[:, :], in_=eb[:])
                        ebs.append(eb)
                    if k + 1 < n_k:
                        Ss = emit_scores(k + 1)
                    if pending_out:
                        emit_outproj(*pending_out.pop(0))
                    for c in range(2):
                        eb = ebs[c]
                        for e_i in range(2):
                            h = 2 * c + e_i
                            # per-region accumulation groups (col-tiled)
                            nc.tensor.matmul(
                                Y[c][64 * e_i:64 * e_i + 64, ytrim:TB],
                                lhsT=v_r[k][:, 64 * h:64 * h + 64],
                                rhs=eb[:, e_i * TB + ytrim:(e_i + 1) * TB],
                                start=st, stop=sp,
                                tile_position=(0, 64 * e_i))
                            nc.tensor.matmul(
                                den32[32 * h:32 * h + 32, ytrim:TB],
                                lhsT=ones32[:],
                                rhs=eb[:, e_i * TB + ytrim:(e_i + 1) * TB],
                                start=st, stop=sp,
                                tile_position=(0, 32 * h))
                # ---- normalize + copy out to yT ----
                invf = tr.tile([P, TB], f32, tag="invf", name=f"invf{j}")
                nc.vector.reciprocal_approx_fast(invf[:], den32[:])
                if _KDBG and j == 0:
                    nc.sync.dma_start(out=dbg_inv0[:, :], in_=invf[:])
                invden = tr.tile([P, TB], f32r, tag="invden", name=f"invden{j}")
                nc.vector.tensor_copy(invden[:], invf[:])
                for c in range(2):
                    bcp = ps.tile([P, TB], f32, tag="tail", name=f"bcp{c}_{j}", bufs=1)
                    nc.tensor.matmul(bcp[:], lhsT=selpair_r[:, c * P:(c + 1) * P],
                                     rhs=invden[:], start=True, stop=True)
                    inv64 = tr.tile([P, TB], f32, tag="inv64", name=f"inv64{c}_{j}")
                    nc.vector.tensor_copy(inv64[:], bcp[:])
                    nc.vector.tensor_mul(yT[c][:, jsl], Y[c][:], inv64[:])
                # queue this row's out-projection for the next row's loop
                pending_out.extend((j, o) for o in range(8))
            while pending_out:
                emit_outproj(*pending_out.pop(0))
            if _KDBG:
                nc.sync.dma_start(out=dbg_yT0[:, :], in_=yT[0][:])

    nc.compile()
    return nc


def _get_module():
    if "nc" not in _CACHE:
        _CACHE["nc"] = _build_module()
        _CACHE["consts"] = _build_consts()
    return _CACHE["nc"], _CACHE["consts"]


def _core_inputs(x, w_q, w_k, w_v, w_proj, core):
    b = core // 4
    g = core % 4
    heads = [4 * g + j for j in range(HPC)]

    xt = _to_bf16_bits(np.ascontiguousarray(x[b].T))      # [C, T]

    perm = np.empty(256, dtype=np.int64)
    for m in range(128):
        perm[m] = 64 * heads[m // 32] + (m % 32)             # x1 half
        perm[128 + m] = 64 * heads[m // 32] + 32 + (m % 32)  # x2 half
    wq = _to_bf16_bits(np.ascontiguousarray(w_q[perm, :].T))  # [C, 256]
    wk = _to_bf16_bits(np.ascontiguousarray(w_k[perm, :].T))

    vperm = np.empty(256, dtype=np.int64)
    for m in range(256):
        vperm[m] = 64 * heads[m // 64] + (m % 64)
    wv = _to_bf16_bits(np.ascontiguousarray(w_v[vperm, :].T))  # [C, 256]
    wp = _to_bf16_bits(np.ascontiguousarray(w_proj[:, vperm].T))  # [256, C]

    return dict(xt=xt, wq=wq, wk=wk, wv=wv, wp=wp)


def kernel(x, w_q, w_k, w_v, w_proj, _trace=False, _trace_cores=None):
    from concourse.bass_utils import run_bass_kernel_spmd

    nc, consts = _get_module()
    x = np.asarray(x, dtype=np.float32)
    in_maps = []
    for core in range(N_CORES):
        m = _core_inputs(np.asarray(x), np.asarray(w_q), np.asarray(w_k),
                         np.asarray(w_v), np.asarray(w_proj), core)
        m.update(consts)
        in_maps.append(m)

    res = run_bass_kernel_spmd(nc, in_maps, list(range(N_CORES)),
                               trace=_trace, trace_cores=_trace_cores)
    outs = []
    for c in range(N_CORES):
        o = np.asarray(res.results[c]["outT"]).astype(np.float32)
        outs.append(o)
    out = np.empty((B, T, C), dtype=np.float32)
    for b in range(B):
        acc = outs[4 * b].astype(np.float32)
        for g in range(1, 4):
            acc = acc + outs[4 * b + g]
        out[b] = acc.T
    if _trace:
        kernel._last_exec_time_ns = res.exec_time_ns
        kernel._last_results = res
    return out
